# revision 18
# baseline (speedup 1.0000x reference)
"""Trainium2 Bass kernel for nn_Batch_Edge (gnn_message_passing).

Computation (see reference):
    node_embed = last_node_batch @ W_embed + b_embed          # [B, H]
    stack      = concat([h, node_embed[seg]], axis=1)         # [N, 2H]
    out        = tanh(stack @ W1 + b1); out = tanh(out @ W2 + b2)
    edges      = out @ W3 + b3                                # [N, 2]
    result     = edges reshaped to [B, max_nodes*2]  (no padding: graphs full)

Strategy: 512 graphs (131072 nodes) sharded contiguously across 8 cores.
Activations feature-on-partition ([feature, node]); h arrives pre-transposed
bf16. The per-graph embedding contribution C = node_embed @ W1[H:, :] + b1
is host-computed in fp64.

The kernel is ACT(tanh)-bound at ~0.83ns/elem, so elementwise work is split
between ACT and DVE (GpSimd cannot touch PSUM, so it cannot help):
  - ACT: tanh for the L1 m0 half (SBUF, FD2048) + all of L2 (PSUM, FD1024,
    b2 as fused bias).
  - DVE: C-add for L1-m0 (TT broadcast from PSUM) + a custom fused DVE op
    TANH5C_ANT (clamp to +-a, degree-5 odd polynomial, 8 ALU stages, ONE
    instruction at ~1ns/elem) for the L1 m1 half, reading PSUM directly.
  - PE: besides the three matmul layers, the m1-half C broadcast is folded
    into the L1 PSUM accumulation with K=2 matmuls (two C columns as
    weights x a 2x512 indicator-row constant), so no separate C-add pass
    is needed on the m1 lane.
The deg-5 poly tanh (max err 1.4e-2 vs tanh) on one quarter of activations
keeps end-to-end rel err ~1.4e-2, under the 2e-2 gate.
"""

import os
import numpy as np

B = 512
NPG = 256               # nodes per graph
N = B * NPG             # 131072
HID = 128
NCORES = 8
GPC = B // NCORES       # 64 graphs per core
NPC = N // NCORES       # 16384 nodes per core
PAD_VALUE = -10000.0

ST = 2048               # supertile: nodes handled per main-loop iteration
NST = NPC // ST         # 8 supertiles per core

# tanh(x) ~= x' * (c2 t^2 + c1 t + c0), x' = clamp(x, +-A5), t = x'^2
# (LP minimax fit incl. saturation; max abs err 1.38e-2)
A5 = 2.140
C5 = (0.9459592131482503, -0.19428502676458773, 0.01929969961895106)
# deg-7 variant for the 2-instruction chain (max abs err 8.1e-3)
A7 = 2.420
C7 = (0.9658507348190916, -0.2314770592167479, 0.036329766992213804,
      -0.0022223059386500313)

# which lanes use the DVE poly instead of ACT tanh
POLY_L1M1 = True        # L1 m1 half -> TANH5C on DVE
POLY_MODE = int(os.environ.get("POLY_MODE", "1"))
POLY_L2M1 = False       # L2 m1 half -> CLAMPB+TANH7P chain on DVE

LAST_RESULT = None      # BassKernelResults of the most recent device run
_CACHE = {}


def _numpy_ref(last_node_batch, h, W_embed, b_embed, W1, b1, W2, b2, W3, b3,
               segment_ids, max_nodes):
    """Exact host fallback (used only if inputs don't match the expected
    uniform-graph structure)."""
    lnb = np.asarray(last_node_batch, np.float32)
    h = np.asarray(h, np.float32)
    seg = np.asarray(segment_ids).astype(np.int64)
    b = lnb.shape[0]
    n = h.shape[0]
    mn = int(np.asarray(max_nodes))
    node_embed = lnb @ np.asarray(W_embed, np.float32) + np.asarray(b_embed, np.float32)
    stack = np.concatenate([h, node_embed[seg]], axis=1)
    out = np.tanh(stack @ np.asarray(W1, np.float32) + np.asarray(b1, np.float32))
    out = np.tanh(out @ np.asarray(W2, np.float32) + np.asarray(b2, np.float32))
    edges = out @ np.asarray(W3, np.float32) + np.asarray(b3, np.float32)
    counts = np.zeros(b, np.int64)
    np.add.at(counts, seg, 1)
    offsets = np.cumsum(counts) - counts
    pos = np.arange(n) - offsets[seg]
    padded = np.full((b, mn, 2), PAD_VALUE, np.float32)
    padded[seg, pos] = edges
    return padded.reshape(b, mn * 2)


def _register_ops():
    """Register the custom fused DVE ops (idempotent)."""
    if "ops" in _CACHE:
        return _CACHE["ops"]
    import concourse.dve_ops as dvo
    from concourse.dve_ops import DveOp
    from concourse.dve_spec import (Spec, Src0, C0, C1, C2, C3, Zero,
                                    minn, maxx, sq, lower, _has_src1,
                                    _spill_c3_to_src1)
    from concourse.dve_uop import DveOpSpec

    def register(name, spec):
        if name in dvo._SUB_OPCODE_FOR_NAME:
            return next(o for o in dvo.OPS if o.name == name)
        row = max(dvo._SUB_OPCODE_FOR_NAME.values()) + 1
        assert row < 0x20
        dvo._SUB_OPCODE_FOR_NAME[name] = row
        uops = lower(spec, ver="v3")
        s = DveOpSpec(name=name, opcode=row, uops=uops, rd1_en=_has_src1(spec))
        op = DveOp(name, spec, subdim=False, uops_sha={"v3": s.sha("v3")})
        dvo.OPS.append(op)
        dvo.CUSTOM_DVE_SPECS[name] = spec
        return op

    # TANH5C: y = x'*((t*c2 + c1)*t + c0), x' = clamp(x, +-a), t = x'^2
    #   s0 = a, s1 = c2, imm2 = c1, in1[P,1] = c0
    def ref_tanh5(in0, in1, c0, c1, c2):
        a = np.asarray(c0, np.float32)
        xp = np.clip(in0.astype(np.float32), -a, a)
        t = xp * xp
        cc0 = np.asarray(in1, np.float32).reshape(-1, 1)
        return xp * ((t * np.float32(c1) + np.float32(c2)) * t + cc0)

    _xc = maxx(minn(Src0, C0), Zero - C0)
    _t5 = sq(_xc)
    op5 = register("TANH5C_ANT", Spec(
        body=_spill_c3_to_src1(((_t5 * C1 + C2) * _t5 + C3) * _xc),
        reference=ref_tanh5))

    # CLAMPB: out = clamp(Src0 + C0, +-C1); s0 = bias [P,1], s1 = a
    def ref_clampb(in0, in1, c0, c1, c2):
        return np.clip(in0.astype(np.float32) + np.asarray(c0, np.float32),
                       -c1, c1)

    op_cb = register("CLAMPB_ANT", Spec(
        body=maxx(minn(Src0 + C0, C1), Zero - C1), reference=ref_clampb))

    # TANH7P: y = x*(((t*c3 + c2)*t + c1)*t + c0), t = x^2 (input preclamped)
    #   s0 = c3, s1 = c2, imm2 = c1, in1[P,1] = c0
    def ref_tanh7(in0, in1, c0, c1, c2):
        x = in0.astype(np.float32)
        t = x * x
        cc0 = np.asarray(in1, np.float32).reshape(-1, 1)
        return x * (((t * np.asarray(c0, np.float32) + np.float32(c1)) * t
                     + np.float32(c2)) * t + cc0)

    _t7 = sq(Src0)
    op7 = register("TANH7P_ANT", Spec(
        body=_spill_c3_to_src1((((_t7 * C0 + C1) * _t7 + C2) * _t7 + C3)
                               * Src0),
        reference=ref_tanh7))

    _CACHE["ops"] = (op5, op_cb, op7)
    return _CACHE["ops"]


def _build():
    """Build + compile the per-core Bass program (identical on all cores)."""
    import concourse.bacc as bacc
    import concourse.mybir as mybir
    import concourse.tile as tile

    op5, op_cb, op7 = _register_ops()

    f32 = mybir.dt.float32
    bf16 = mybir.dt.bfloat16
    Tanh = mybir.ActivationFunctionType.Tanh
    Add = mybir.AluOpType.add

    nc = bacc.Bacc("TRN2", target_bir_lowering=False, debug=False, enable_asserts=False)

    # wpk layout (free dim): w1t[0:256] w2a[256:512] w2b[512:768]
    #                        w3a[768:770] w3b[770:772]
    hT = nc.dram_tensor("hT", [128, NPC], bf16, kind="ExternalInput").ap()
    wpk = nc.dram_tensor("wpk", [128, 772], bf16, kind="ExternalInput").ap()
    # bpk columns: 0 b2a, 1 b2b, 2:66 C^T m0, 66:130 C^T m1,
    #              130 poly c0 (deg5), 131 poly c0 (deg7)
    bpk = nc.dram_tensor("bpk", [128, 132], f32, kind="ExternalInput").ap()
    # cpk rows 0-1 (padded to K=128 so the C-fold matmul uses the same PE
    # tile config as the main matmul in its accumulation group):
    # [*, 0:512] indicator rows (1 for own graph's 256 nodes),
    # [*, 512 + ci*128 : +128] = C^T m1 for chunk ci's 2 graphs; rows 2+ zero.
    cpk = nc.dram_tensor("cpk", [128, 512 + (GPC // 2) * 128], bf16,
                         kind="ExternalInput").ap()
    # out rows {32*jj + c}: [32*jj + c, st*512 + k] = edges[c, st*2048 + jj*512 + k]
    # (other rows are garbage; one wide DMA per supertile beats 4 narrow ones)
    out_d = nc.dram_tensor("out", [98, NPC // 4], bf16, kind="ExternalOutput").ap()

    with tile.TileContext(nc) as tc:
        with (
            tc.tile_pool(name="w", bufs=1) as wp,
            tc.tile_pool(name="io", bufs=2) as io,
            tc.tile_pool(name="act", bufs=2) as ac,
            tc.tile_pool(name="ps1", bufs=4, space="PSUM") as ps1,
            tc.tile_pool(name="ps2", bufs=2, space="PSUM") as ps2,
        ):
            # biases + host-computed C first (tiny DMA)
            s_b = wp.tile([128, 132], f32, tag="bpk")
            nc.sync.dma_start(out=s_b[:], in_=bpk)
            s_cp = wp.tile([128, 512 + (GPC // 2) * 128], bf16, tag="cpk")
            nc.sync.dma_start(out=s_cp[:], in_=cpk)
            # L1 weights next, then h chunk 0, then the rest
            s_w = wp.tile([128, 772], bf16, tag="wpk")
            nc.sync.dma_start(out=s_w[:, 0:256], in_=wpk[:, 0:256])
            h_tiles = {}
            t_h0 = io.tile([128, ST], bf16, tag="h")
            nc.sync.dma_start(out=t_h0[:, 0:ST // 2], in_=hT[:, 0:ST // 2])
            nc.sync.dma_start(out=t_h0[:, ST // 2:ST], in_=hT[:, ST // 2:ST])
            h_tiles[0] = t_h0
            nc.sync.dma_start(out=s_w[:, 256:772], in_=wpk[:, 256:772])
            s_w1t = s_w[:, 0:256]
            s_w2a = s_w[:, 256:512]
            s_w2b = s_w[:, 512:768]
            s_w3a = s_w[:, 768:770]
            s_w3b = s_w[:, 770:772]
            s_b2 = [s_b[:, 0:1], s_b[:, 1:2]]
            s_ct = [s_b[:, 2:66], s_b[:, 66:130]]
            s_c05 = s_b[:, 130:131]
            s_c07 = s_b[:, 131:132]

            for st in range(NST):
                if st in h_tiles:
                    t_h = h_tiles.pop(st)
                else:
                    t_h = io.tile([128, ST], bf16, tag="h")
                    nc.sync.dma_start(
                        out=t_h[:], in_=hT[:, st * ST:(st + 1) * ST],
                    )

                # ---- L1: z1[m] = W1[:H, m].T @ h^T ; +C per-graph ----
                # m0: DVE TT C-add -> SBUF bf16, ACT tanh FD2048.
                # m1: C folded into PSUM by a K=2 indicator matmul on PE;
                #     DVE TANH5C (fused clamp + deg-5 poly tanh, one
                #     instruction) evacuates PSUM -> bf16 SBUF directly.
                y1 = []
                for m in (0, 1):
                    y1t = ac.tile([128, ST], bf16, tag=f"y1{m}")
                    if m == 1 and POLY_L1M1 and POLY_MODE in (1, 3):
                        if POLY_MODE == 3:
                            y1x = ac.tile([128, ST], bf16, tag="y1x")
                        else:
                            y1x = None
                        for j in range(ST // 512):
                            p1 = ps1.tile([128, 512], f32, tag="ps1")
                            nc.tensor.matmul(
                                p1[:], s_w1t[:, 128:256],
                                t_h[:, 512 * j:512 * j + 512],
                                start=True, stop=False,
                            )
                            ci = st * (ST // 512) + j
                            nc.tensor.matmul(
                                p1[:], s_cp[:, 512 + ci * 128:512 + ci * 128 + 128],
                                s_cp[:, 0:512],
                                start=False, stop=True,
                            )
                            if POLY_MODE == 1:
                                nc.vector._custom_dve(
                                    op5, out=y1t[:, 512 * j:512 * j + 512],
                                    in0=p1[:], in1=s_c05,
                                    s0=A5, s1=C5[2], imm2=C5[1],
                                )
                            else:
                                nc.vector.tensor_copy(
                                    y1x[:, 512 * j:512 * j + 512], p1[:])
                        if POLY_MODE == 3:
                            nc.scalar.activation(y1t[:], y1x[:], Tanh)
                        y1.append(y1t)
                        continue
                    if m == 1 and POLY_L1M1 and POLY_MODE == 2:
                        y1s = ac.tile([128, ST], bf16, tag="y1s1")
                        for j in range(ST // 512):
                            p1 = ps1.tile([128, 512], f32, tag="ps1")
                            nc.tensor.matmul(
                                p1[:], s_w1t[:, 128:256],
                                t_h[:, 512 * j:512 * j + 512],
                                start=True, stop=True,
                            )
                            g = st * (ST // NPG) + j * 2
                            nc.vector.tensor_tensor(
                                y1s[:, 512 * j:512 * j + 512]
                                .rearrange("p (a b) -> p a b", a=2),
                                p1[:].rearrange("p (a b) -> p a b", a=2),
                                s_ct[1][:, g:g + 2].broadcast_to((128, 2, 256)),
                                Add,
                            )
                        nc.vector._custom_dve(
                            op5, out=y1t[:], in0=y1s[:], in1=s_c05,
                            s0=A5, s1=C5[2], imm2=C5[1],
                        )
                        y1.append(y1t)
                        continue
                    y1s = ac.tile([128, ST], bf16, tag=f"y1s{m}")
                    for j in range(ST // 512):
                        p1 = ps1.tile([128, 512], f32, tag="ps1")
                        nc.tensor.matmul(
                            p1[:], s_w1t[:, 128 * m:128 * m + 128],
                            t_h[:, 512 * j:512 * j + 512],
                            start=True, stop=True,
                        )
                        g = st * (ST // NPG) + j * 2
                        nc.vector.tensor_tensor(
                            y1s[:, 512 * j:512 * j + 512]
                            .rearrange("p (a b) -> p a b", a=2),
                            p1[:].rearrange("p (a b) -> p a b", a=2),
                            s_ct[m][:, g:g + 2].broadcast_to((128, 2, 256)),
                            Add,
                        )
                    if st == 0 and m == 0:
                        # first supertile: FD=512 slices so the Scalar queue
                        # saturates earlier out of the DMA head
                        for j in range(ST // 512):
                            nc.scalar.activation(
                                y1t[:, 512 * j:512 * j + 512],
                                y1s[:, 512 * j:512 * j + 512], Tanh,
                            )
                    else:
                        nc.scalar.activation(y1t[:], y1s[:], Tanh)
                    y1.append(y1t)

                # ---- L2: z2[m] = W2[:, m].T @ y1 (+b2) ----
                # m0 (and m1 unless POLY_L2M1): ACT tanh straight from PSUM
                # with b2 as fused bias, FD1024.
                y2 = []
                for m in (0, 1):
                    yt = ac.tile([128, ST], bf16, tag=f"y2{m}")
                    use_poly = (m == 1 and POLY_L2M1)
                    xc = ac.tile([128, ST], bf16, tag="xc") if use_poly else None
                    for jj in range(ST // 1024):
                        p2 = ps2.tile([128, 1024], f32, tag="ps2")
                        for j2 in (0, 1):
                            sl = 1024 * jj + 512 * j2
                            po = 512 * j2
                            nc.tensor.matmul(
                                p2[:, po:po + 512],
                                s_w2a[:, 128 * m:128 * m + 128],
                                y1[0][:, sl:sl + 512],
                                start=True, stop=False,
                            )
                            nc.tensor.matmul(
                                p2[:, po:po + 512],
                                s_w2b[:, 128 * m:128 * m + 128],
                                y1[1][:, sl:sl + 512],
                                start=False, stop=True,
                            )
                        if use_poly:
                            nc.vector._custom_dve(
                                op_cb, out=xc[:, 1024 * jj:1024 * jj + 1024],
                                in0=p2[:], in1=None, s0=s_b2[1], s1=A7,
                            )
                        else:
                            nc.scalar.activation(
                                yt[:, 1024 * jj:1024 * jj + 1024], p2[:],
                                Tanh, bias=s_b2[m],
                            )
                    if use_poly:
                        nc.vector._custom_dve(
                            op7, out=yt[:], in0=xc[:], in1=s_c07,
                            s0=C7[3], s1=C7[2], imm2=C7[1],
                        )
                    y2.append(yt)

                # ---- L3: edges^T = W3a.T @ y2a + W3b.T @ y2b (M=2) ----
                # 4-way PE column tiling: chunk jj lands in PSUM partitions
                # [32jj, 32jj+2) of ONE bank; one [98, 512] copy on the Pool
                # engine (cost is free-dim-bound) evacuates all four pairs.
                p3 = ps1.tile([128, 512], f32, tag="ps1")
                for jj in range(4):
                    nc.tensor.matmul(
                        p3[32 * jj:32 * jj + 2, :], s_w3a,
                        y2[0][:, 512 * jj:512 * jj + 512],
                        start=True, stop=False, tile_position=(0, 32 * jj),
                    )
                for jj in range(4):
                    nc.tensor.matmul(
                        p3[32 * jj:32 * jj + 2, :], s_w3b,
                        y2[1][:, 512 * jj:512 * jj + 512],
                        start=False, stop=True, tile_position=(0, 32 * jj),
                    )
                ed = io.tile([98, 512], bf16, tag="ed")
                nc.vector.tensor_copy(ed[:], p3[0:98, :])
                nc.sync.dma_start(
                    out=out_d[:, st * 512:(st + 1) * 512], in_=ed[:],
                )

    nc.compile()
    return nc


def kernel(last_node_batch, h, W_embed, b_embed, W1, b1, W2, b2, W3, b3,
           segment_ids, max_nodes):
    global LAST_RESULT
    lnb = np.asarray(last_node_batch, np.float32)
    h = np.asarray(h, np.float32)
    seg = np.asarray(segment_ids)
    mn = int(np.asarray(max_nodes))

    expected_seg = np.repeat(np.arange(B, dtype=seg.dtype), NPG)
    if not (lnb.shape == (B, HID) and h.shape == (N, HID) and mn == NPG
            and seg.shape == (N,) and np.array_equal(seg, expected_seg)):
        return _numpy_ref(last_node_batch, h, W_embed, b_embed, W1, b1, W2, b2,
                          W3, b3, segment_ids, max_nodes)

    import sys
    try:
        import antenv.axon_hooks  # noqa: F401
    except ImportError:
        # bass_utils imports this unconditionally when tracing is requested
        # (e.g. BASS_TRACE set in the environment); provide a no-op fallback
        # so tracing degrades instead of crashing.
        import types
        _m = types.ModuleType("antenv.axon_hooks")
        _m.get_axon_ntff_profile_hook = lambda: None
        _m.set_axon_ntff_profile_hook = lambda h: None
        sys.modules["antenv.axon_hooks"] = _m

    import ml_dtypes
    from concourse.bass_utils import run_bass_kernel_spmd

    bf16 = ml_dtypes.bfloat16

    if "nc" not in _CACHE:
        _CACHE["nc"] = _build()
    nc = _CACHE["nc"]

    W1 = np.asarray(W1, np.float32)
    W2 = np.asarray(W2, np.float32)
    W3 = np.asarray(W3, np.float32)
    b2v = np.asarray(b2, np.float32)
    b3v = np.asarray(b3, np.float32)

    # Per-graph contribution C = (lnb @ W_embed + b_embed) @ W1[H:] + b1,
    # computed on host in fp64.
    emb = lnb.astype(np.float64) @ np.asarray(W_embed, np.float64) \
        + np.asarray(b_embed, np.float64)
    C = (emb @ W1[HID:, :].astype(np.float64)
         + np.asarray(b1, np.float64)).astype(np.float32)   # [B, 2H]

    wpk = np.ascontiguousarray(np.concatenate([
        W1[:HID, :].astype(bf16),
        W2[:HID, :].astype(bf16), W2[HID:, :].astype(bf16),
        W3[:HID, :].astype(bf16), W3[HID:, :].astype(bf16),
    ], axis=1))

    # indicator rows for the K=128(padded) C-fold matmul: row p in {0,1}
    # covers nodes of graph 2ci+p within a 512-node chunk
    ind = np.zeros((128, 512), np.float32)
    ind[0, :256] = 1.0
    ind[1, 256:] = 1.0

    in_maps = []
    for c in range(NCORES):
        Cc = C[c * GPC:(c + 1) * GPC]                       # [64, 256]
        bpk = np.concatenate([
            b2v[:HID, None], b2v[HID:, None],
            np.ascontiguousarray(Cc[:, :HID].T),
            np.ascontiguousarray(Cc[:, HID:].T),
            np.full((HID, 1), C5[0], np.float32),
            np.full((HID, 1), C7[0], np.float32),
        ], axis=1)
        # cpk: [2, 512] indicator, then per 2-graph chunk ci the two C-m1
        # rows as a [2, 128] block
        cm1 = Cc[:, HID:].reshape(GPC // 2, 2, HID)          # [32, 2, 128]
        cblk = np.zeros((128, (GPC // 2) * HID), np.float32)
        cblk[0:2] = cm1.transpose(1, 0, 2).reshape(2, -1)
        cpk = np.concatenate([ind, cblk], axis=1)
        m = {
            "wpk": wpk,
            "bpk": np.ascontiguousarray(bpk),
            "cpk": np.ascontiguousarray(cpk).astype(bf16),
            "hT": np.ascontiguousarray(h[c * NPC:(c + 1) * NPC].T).astype(bf16),
        }
        in_maps.append(m)

    trace = bool(int(os.environ.get("KERNEL_TRACE", "0")))
    res = run_bass_kernel_spmd(nc, in_maps, core_ids=list(range(NCORES)),
                               trace=trace)
    LAST_RESULT = res

    out = np.empty((B, NPG * 2), np.float32)
    for c in range(NCORES):
        od = res.results[c]["out"]          # [98, 4096] bf16; rows 32*jj+cc live
        sel = od[[0, 1, 32, 33, 64, 65, 96, 97]].astype(np.float32)
        # sel[2*jj + cc, blk*512 + k] = edges[cc, blk*2048 + jj*512 + k]
        e = sel.reshape(4, 2, NPC // 2048, 512).transpose(1, 2, 0, 3).reshape(2, NPC)
        blk = e.reshape(2, GPC, NPG).transpose(1, 2, 0).reshape(GPC, NPG * 2)
        out[c * GPC:(c + 1) * GPC] = blk
    out += np.tile(b3v, NPG)[None, :]
    return out


# revision 19
# speedup vs baseline: 1.1345x; 1.1345x over previous
"""Trainium2 Bass kernel for nn_Batch_Edge (gnn_message_passing).

Computation (see reference):
    node_embed = last_node_batch @ W_embed + b_embed          # [B, H]
    stack      = concat([h, node_embed[seg]], axis=1)         # [N, 2H]
    out        = tanh(stack @ W1 + b1); out = tanh(out @ W2 + b2)
    edges      = out @ W3 + b3                                # [N, 2]
    result     = edges reshaped to [B, max_nodes*2]  (no padding: graphs full)

Strategy: 512 graphs (131072 nodes) sharded contiguously across 8 cores.
Activations feature-on-partition ([feature, node]); h arrives pre-transposed
bf16. The per-graph embedding contribution C = node_embed @ W1[H:, :] + b1
is host-computed in fp64.

The kernel is ACT(tanh)-bound at ~0.83ns/elem, so elementwise work is split
between ACT and DVE (GpSimd cannot touch PSUM, so it cannot help):
  - ACT: tanh for the L1 m0 half (SBUF, FD2048) + all of L2 (PSUM, FD1024,
    b2 as fused bias).
  - DVE: C-add for L1-m0 (TT broadcast from PSUM) + a custom fused DVE op
    TANH5C_ANT (clamp to +-a, degree-5 odd polynomial, 8 ALU stages, ONE
    instruction at ~1ns/elem) for the L1 m1 half, reading PSUM directly.
  - PE: besides the three matmul layers, the m1-half C broadcast is folded
    into the L1 PSUM accumulation with K=2 matmuls (two C columns as
    weights x a 2x512 indicator-row constant), so no separate C-add pass
    is needed on the m1 lane.
The deg-5 poly tanh (max err 1.4e-2 vs tanh) on one quarter of activations
keeps end-to-end rel err ~1.4e-2, under the 2e-2 gate.
"""

import os
import numpy as np

B = 512
NPG = 256               # nodes per graph
N = B * NPG             # 131072
HID = 128
NCORES = 8
GPC = B // NCORES       # 64 graphs per core
NPC = N // NCORES       # 16384 nodes per core
PAD_VALUE = -10000.0

ST = 2048               # supertile: nodes handled per main-loop iteration
NST = NPC // ST         # 8 supertiles per core

# tanh(x) ~= x' * (c2 t^2 + c1 t + c0), x' = clamp(x, +-A5), t = x'^2
# (LP minimax fit incl. saturation; max abs err 1.38e-2)
A5 = 2.140
C5 = (0.9459592131482503, -0.19428502676458773, 0.01929969961895106)
# deg-7 variant for the 2-instruction chain (max abs err 8.1e-3)
A7 = 2.420
C7 = (0.9658507348190916, -0.2314770592167479, 0.036329766992213804,
      -0.0022223059386500313)

# which lanes use the DVE poly instead of ACT tanh
POLY_L1M1 = True        # L1 m1 half -> TANH5C on DVE
POLY_MODE = int(os.environ.get("POLY_MODE", "1"))
POLY_L2M1 = False       # L2 m1 half -> CLAMPB+TANH7P chain on DVE

LAST_RESULT = None      # BassKernelResults of the most recent device run
_CACHE = {}


def _numpy_ref(last_node_batch, h, W_embed, b_embed, W1, b1, W2, b2, W3, b3,
               segment_ids, max_nodes):
    """Exact host fallback (used only if inputs don't match the expected
    uniform-graph structure)."""
    lnb = np.asarray(last_node_batch, np.float32)
    h = np.asarray(h, np.float32)
    seg = np.asarray(segment_ids).astype(np.int64)
    b = lnb.shape[0]
    n = h.shape[0]
    mn = int(np.asarray(max_nodes))
    node_embed = lnb @ np.asarray(W_embed, np.float32) + np.asarray(b_embed, np.float32)
    stack = np.concatenate([h, node_embed[seg]], axis=1)
    out = np.tanh(stack @ np.asarray(W1, np.float32) + np.asarray(b1, np.float32))
    out = np.tanh(out @ np.asarray(W2, np.float32) + np.asarray(b2, np.float32))
    edges = out @ np.asarray(W3, np.float32) + np.asarray(b3, np.float32)
    counts = np.zeros(b, np.int64)
    np.add.at(counts, seg, 1)
    offsets = np.cumsum(counts) - counts
    pos = np.arange(n) - offsets[seg]
    padded = np.full((b, mn, 2), PAD_VALUE, np.float32)
    padded[seg, pos] = edges
    return padded.reshape(b, mn * 2)


def _register_ops():
    """Register the custom fused DVE ops (idempotent)."""
    if "ops" in _CACHE:
        return _CACHE["ops"]
    import concourse.dve_ops as dvo
    from concourse.dve_ops import DveOp
    from concourse.dve_spec import (Spec, Src0, C0, C1, C2, C3, Zero,
                                    minn, maxx, sq, lower, _has_src1,
                                    _spill_c3_to_src1)
    from concourse.dve_uop import DveOpSpec

    def register(name, spec):
        if name in dvo._SUB_OPCODE_FOR_NAME:
            return next(o for o in dvo.OPS if o.name == name)
        row = max(dvo._SUB_OPCODE_FOR_NAME.values()) + 1
        assert row < 0x20
        dvo._SUB_OPCODE_FOR_NAME[name] = row
        uops = lower(spec, ver="v3")
        s = DveOpSpec(name=name, opcode=row, uops=uops, rd1_en=_has_src1(spec))
        op = DveOp(name, spec, subdim=False, uops_sha={"v3": s.sha("v3")})
        dvo.OPS.append(op)
        dvo.CUSTOM_DVE_SPECS[name] = spec
        return op

    # TANH5C: y = x'*((t*c2 + c1)*t + c0), x' = clamp(x, +-a), t = x'^2
    #   s0 = a, s1 = c2, imm2 = c1, in1[P,1] = c0
    def ref_tanh5(in0, in1, c0, c1, c2):
        a = np.asarray(c0, np.float32)
        xp = np.clip(in0.astype(np.float32), -a, a)
        t = xp * xp
        cc0 = np.asarray(in1, np.float32).reshape(-1, 1)
        return xp * ((t * np.float32(c1) + np.float32(c2)) * t + cc0)

    _xc = maxx(minn(Src0, C0), Zero - C0)
    _t5 = sq(_xc)
    op5 = register("TANH5C_ANT", Spec(
        body=_spill_c3_to_src1(((_t5 * C1 + C2) * _t5 + C3) * _xc),
        reference=ref_tanh5))

    # CLAMPB: out = clamp(Src0 + C0, +-C1); s0 = bias [P,1], s1 = a
    def ref_clampb(in0, in1, c0, c1, c2):
        return np.clip(in0.astype(np.float32) + np.asarray(c0, np.float32),
                       -c1, c1)

    op_cb = register("CLAMPB_ANT", Spec(
        body=maxx(minn(Src0 + C0, C1), Zero - C1), reference=ref_clampb))

    # TANH7P: y = x*(((t*c3 + c2)*t + c1)*t + c0), t = x^2 (input preclamped)
    #   s0 = c3, s1 = c2, imm2 = c1, in1[P,1] = c0
    def ref_tanh7(in0, in1, c0, c1, c2):
        x = in0.astype(np.float32)
        t = x * x
        cc0 = np.asarray(in1, np.float32).reshape(-1, 1)
        return x * (((t * np.asarray(c0, np.float32) + np.float32(c1)) * t
                     + np.float32(c2)) * t + cc0)

    _t7 = sq(Src0)
    op7 = register("TANH7P_ANT", Spec(
        body=_spill_c3_to_src1((((_t7 * C0 + C1) * _t7 + C2) * _t7 + C3)
                               * Src0),
        reference=ref_tanh7))

    _CACHE["ops"] = (op5, op_cb, op7)
    return _CACHE["ops"]


def _build():
    """Build + compile the per-core Bass program (identical on all cores)."""
    import concourse.bacc as bacc
    import concourse.mybir as mybir
    import concourse.tile as tile

    op5, op_cb, op7 = _register_ops()

    f32 = mybir.dt.float32
    bf16 = mybir.dt.bfloat16
    Tanh = mybir.ActivationFunctionType.Tanh
    Add = mybir.AluOpType.add

    nc = bacc.Bacc("TRN2", target_bir_lowering=False, debug=False, enable_asserts=False)

    # wpk layout (free dim): w1t[0:256] w2a[256:512] w2b[512:768]
    #                        w3a[768:770] w3b[770:772]
    hT = nc.dram_tensor("hT", [128, NPC], bf16, kind="ExternalInput").ap()
    wpk = nc.dram_tensor("wpk", [128, 772], bf16, kind="ExternalInput").ap()
    # bpk columns: 0 b2a, 1 b2b, 2:66 C^T m0, 66:130 C^T m1,
    #              130 poly c0 (deg5), 131 poly c0 (deg7)
    bpk = nc.dram_tensor("bpk", [128, 132], f32, kind="ExternalInput").ap()
    # cpk rows 0-1 (padded to K=128 so the C-fold matmul uses the same PE
    # tile config as the main matmul in its accumulation group):
    # [*, 0:512] indicator rows (1 for own graph's 256 nodes),
    # [*, 512 + ci*128 : +128] = C^T m1 for chunk ci's 2 graphs; rows 2+ zero.
    cpk = nc.dram_tensor("cpk", [128, 512 + (GPC // 2) * 128], bf16,
                         kind="ExternalInput").ap()
    # out rows {32*jj + c}: [32*jj + c, st*512 + k] = edges[c, st*2048 + jj*512 + k]
    # (other rows are garbage; one wide DMA per supertile beats 4 narrow ones)
    out_d = nc.dram_tensor("out", [98, NPC // 4], bf16, kind="ExternalOutput").ap()

    with tile.TileContext(nc) as tc:
        with (
            tc.tile_pool(name="w", bufs=1) as wp,
            tc.tile_pool(name="io", bufs=2) as io,
            tc.tile_pool(name="act", bufs=2) as ac,
            tc.tile_pool(name="ps1", bufs=3, space="PSUM") as ps1,
            tc.tile_pool(name="ps2", bufs=2, space="PSUM") as ps2,
            tc.tile_pool(name="ps3", bufs=1, space="PSUM") as ps3,
        ):
            # biases + host-computed C first (tiny DMA)
            s_b = wp.tile([128, 132], f32, tag="bpk")
            nc.sync.dma_start(out=s_b[:], in_=bpk)
            s_cp = wp.tile([128, 512 + (GPC // 2) * 128], bf16, tag="cpk")
            nc.sync.dma_start(out=s_cp[:], in_=cpk)
            # L1 weights next, then h chunk 0, then the rest
            s_w = wp.tile([128, 772], bf16, tag="wpk")
            nc.sync.dma_start(out=s_w[:, 0:256], in_=wpk[:, 0:256])
            h_tiles = {}
            t_h0 = io.tile([128, ST], bf16, tag="h")
            nc.sync.dma_start(out=t_h0[:, 0:ST // 2], in_=hT[:, 0:ST // 2])
            nc.sync.dma_start(out=t_h0[:, ST // 2:ST], in_=hT[:, ST // 2:ST])
            h_tiles[0] = t_h0
            nc.sync.dma_start(out=s_w[:, 256:772], in_=wpk[:, 256:772])
            s_w1t = s_w[:, 0:256]
            s_w2a = s_w[:, 256:512]
            s_w2b = s_w[:, 512:768]
            s_w3a = s_w[:, 768:770]
            s_w3b = s_w[:, 770:772]
            s_b2 = [s_b[:, 0:1], s_b[:, 1:2]]
            s_ct = [s_b[:, 2:66], s_b[:, 66:130]]
            s_c05 = s_b[:, 130:131]
            s_c07 = s_b[:, 131:132]

            for st in range(NST):
                if st in h_tiles:
                    t_h = h_tiles.pop(st)
                else:
                    t_h = io.tile([128, ST], bf16, tag="h")
                    nc.sync.dma_start(
                        out=t_h[:], in_=hT[:, st * ST:(st + 1) * ST],
                    )

                # ---- L1: z1[m] = W1[:H, m].T @ h^T ; +C per-graph ----
                # m0: DVE TT C-add -> SBUF bf16, ACT tanh FD2048.
                # m1: C folded into PSUM by a K=2 indicator matmul on PE;
                #     DVE TANH5C (fused clamp + deg-5 poly tanh, one
                #     instruction) evacuates PSUM -> bf16 SBUF directly.
                y1 = []
                for m in (0, 1):
                    y1t = ac.tile([128, ST], bf16, tag=f"y1{m}")
                    if m == 1 and POLY_L1M1 and POLY_MODE in (1, 3):
                        if POLY_MODE == 3:
                            y1x = ac.tile([128, ST], bf16, tag="y1x")
                        else:
                            y1x = None
                        for j in range(ST // 512):
                            p1 = ps1.tile([128, 512], f32, tag="ps1")
                            nc.tensor.matmul(
                                p1[:], s_w1t[:, 128:256],
                                t_h[:, 512 * j:512 * j + 512],
                                start=True, stop=False,
                            )
                            ci = st * (ST // 512) + j
                            nc.tensor.matmul(
                                p1[:], s_cp[:, 512 + ci * 128:512 + ci * 128 + 128],
                                s_cp[:, 0:512],
                                start=False, stop=True,
                            )
                            if POLY_MODE == 1:
                                nc.vector._custom_dve(
                                    op5, out=y1t[:, 512 * j:512 * j + 512],
                                    in0=p1[:], in1=s_c05,
                                    s0=A5, s1=C5[2], imm2=C5[1],
                                )
                            else:
                                nc.vector.tensor_copy(
                                    y1x[:, 512 * j:512 * j + 512], p1[:])
                        if POLY_MODE == 3:
                            nc.scalar.activation(y1t[:], y1x[:], Tanh)
                        y1.append(y1t)
                        continue
                    if m == 1 and POLY_L1M1 and POLY_MODE in (2, 4):
                        y1s = ac.tile([128, ST], bf16, tag="y1s1")
                        for j in range(ST // 512):
                            p1 = ps1.tile([128, 512], f32, tag="ps1")
                            nc.tensor.matmul(
                                p1[:], s_w1t[:, 128:256],
                                t_h[:, 512 * j:512 * j + 512],
                                start=True, stop=True,
                            )
                            g = st * (ST // NPG) + j * 2
                            nc.vector.tensor_tensor(
                                y1s[:, 512 * j:512 * j + 512]
                                .rearrange("p (a b) -> p a b", a=2),
                                p1[:].rearrange("p (a b) -> p a b", a=2),
                                s_ct[1][:, g:g + 2].broadcast_to((128, 2, 256)),
                                Add,
                            )
                        if POLY_MODE == 2:
                            nc.vector._custom_dve(
                                op5, out=y1t[:], in0=y1s[:], in1=s_c05,
                                s0=A5, s1=C5[2], imm2=C5[1],
                            )
                        else:
                            # half poly on DVE, half exact tanh on ACT
                            nc.vector._custom_dve(
                                op5, out=y1t[:, 0:1024], in0=y1s[:, 0:1024],
                                in1=s_c05, s0=A5, s1=C5[2], imm2=C5[1],
                            )
                            nc.scalar.activation(
                                y1t[:, 1024:2048], y1s[:, 1024:2048], Tanh)
                        y1.append(y1t)
                        continue
                    y1s = ac.tile([128, ST], bf16, tag=f"y1s{m}")
                    for j in range(ST // 512):
                        p1 = ps1.tile([128, 512], f32, tag="ps1")
                        nc.tensor.matmul(
                            p1[:], s_w1t[:, 128 * m:128 * m + 128],
                            t_h[:, 512 * j:512 * j + 512],
                            start=True, stop=True,
                        )
                        g = st * (ST // NPG) + j * 2
                        nc.vector.tensor_tensor(
                            y1s[:, 512 * j:512 * j + 512]
                            .rearrange("p (a b) -> p a b", a=2),
                            p1[:].rearrange("p (a b) -> p a b", a=2),
                            s_ct[m][:, g:g + 2].broadcast_to((128, 2, 256)),
                            Add,
                        )
                    if st == 0 and m == 0:
                        # first supertile: FD=512 slices so the Scalar queue
                        # saturates earlier out of the DMA head
                        for j in range(ST // 512):
                            nc.scalar.activation(
                                y1t[:, 512 * j:512 * j + 512],
                                y1s[:, 512 * j:512 * j + 512], Tanh,
                            )
                    else:
                        nc.scalar.activation(y1t[:], y1s[:], Tanh)
                    y1.append(y1t)

                # ---- L2: z2[m] = W2[:, m].T @ y1 (+b2) ----
                # m0 (and m1 unless POLY_L2M1): ACT tanh straight from PSUM
                # with b2 as fused bias, FD1024.
                y2 = []
                for m in (0, 1):
                    yt = ac.tile([128, ST], bf16, tag=f"y2{m}")
                    use_poly = (m == 1 and POLY_L2M1)
                    xc = ac.tile([128, ST], bf16, tag="xc") if use_poly else None
                    for jj in range(ST // 1024):
                        p2 = ps2.tile([128, 1024], f32, tag="ps2")
                        for j2 in (0, 1):
                            sl = 1024 * jj + 512 * j2
                            po = 512 * j2
                            nc.tensor.matmul(
                                p2[:, po:po + 512],
                                s_w2a[:, 128 * m:128 * m + 128],
                                y1[0][:, sl:sl + 512],
                                start=True, stop=False,
                            )
                            nc.tensor.matmul(
                                p2[:, po:po + 512],
                                s_w2b[:, 128 * m:128 * m + 128],
                                y1[1][:, sl:sl + 512],
                                start=False, stop=True,
                            )
                        if use_poly:
                            nc.vector._custom_dve(
                                op_cb, out=xc[:, 1024 * jj:1024 * jj + 1024],
                                in0=p2[:], in1=None, s0=s_b2[1], s1=A7,
                            )
                        else:
                            nc.scalar.activation(
                                yt[:, 1024 * jj:1024 * jj + 1024], p2[:],
                                Tanh, bias=s_b2[m],
                            )
                    if use_poly:
                        nc.vector._custom_dve(
                            op7, out=yt[:], in0=xc[:], in1=s_c07,
                            s0=C7[3], s1=C7[2], imm2=C7[1],
                        )
                    y2.append(yt)

                # ---- L3: edges^T = W3a.T @ y2a + W3b.T @ y2b (M=2) ----
                # 4-way PE column tiling: chunk jj lands in PSUM partitions
                # [32jj, 32jj+2) of ONE bank; one [98, 512] copy on the Pool
                # engine (cost is free-dim-bound) evacuates all four pairs.
                p3 = ps3.tile([128, 512], f32, tag="ps3")
                for jj in range(4):
                    nc.tensor.matmul(
                        p3[32 * jj:32 * jj + 2, :], s_w3a,
                        y2[0][:, 512 * jj:512 * jj + 512],
                        start=True, stop=False, tile_position=(0, 32 * jj),
                    )
                for jj in range(4):
                    nc.tensor.matmul(
                        p3[32 * jj:32 * jj + 2, :], s_w3b,
                        y2[1][:, 512 * jj:512 * jj + 512],
                        start=False, stop=True, tile_position=(0, 32 * jj),
                    )
                ed = io.tile([98, 512], bf16, tag="ed")
                nc.vector.tensor_copy(ed[:], p3[0:98, :])
                nc.sync.dma_start(
                    out=out_d[:, st * 512:(st + 1) * 512], in_=ed[:],
                )

    nc.compile()
    return nc


def kernel(last_node_batch, h, W_embed, b_embed, W1, b1, W2, b2, W3, b3,
           segment_ids, max_nodes):
    global LAST_RESULT
    lnb = np.asarray(last_node_batch, np.float32)
    h = np.asarray(h, np.float32)
    seg = np.asarray(segment_ids)
    mn = int(np.asarray(max_nodes))

    expected_seg = np.repeat(np.arange(B, dtype=seg.dtype), NPG)
    if not (lnb.shape == (B, HID) and h.shape == (N, HID) and mn == NPG
            and seg.shape == (N,) and np.array_equal(seg, expected_seg)):
        return _numpy_ref(last_node_batch, h, W_embed, b_embed, W1, b1, W2, b2,
                          W3, b3, segment_ids, max_nodes)

    import sys
    try:
        import antenv.axon_hooks  # noqa: F401
    except ImportError:
        # bass_utils imports this unconditionally when tracing is requested
        # (e.g. BASS_TRACE set in the environment); provide a no-op fallback
        # so tracing degrades instead of crashing.
        import types
        _m = types.ModuleType("antenv.axon_hooks")
        _m.get_axon_ntff_profile_hook = lambda: None
        _m.set_axon_ntff_profile_hook = lambda h: None
        sys.modules["antenv.axon_hooks"] = _m

    import ml_dtypes
    from concourse.bass_utils import run_bass_kernel_spmd

    bf16 = ml_dtypes.bfloat16

    if "nc" not in _CACHE:
        _CACHE["nc"] = _build()
    nc = _CACHE["nc"]

    W1 = np.asarray(W1, np.float32)
    W2 = np.asarray(W2, np.float32)
    W3 = np.asarray(W3, np.float32)
    b2v = np.asarray(b2, np.float32)
    b3v = np.asarray(b3, np.float32)

    # Per-graph contribution C = (lnb @ W_embed + b_embed) @ W1[H:] + b1,
    # computed on host in fp64.
    emb = lnb.astype(np.float64) @ np.asarray(W_embed, np.float64) \
        + np.asarray(b_embed, np.float64)
    C = (emb @ W1[HID:, :].astype(np.float64)
         + np.asarray(b1, np.float64)).astype(np.float32)   # [B, 2H]

    wpk = np.ascontiguousarray(np.concatenate([
        W1[:HID, :].astype(bf16),
        W2[:HID, :].astype(bf16), W2[HID:, :].astype(bf16),
        W3[:HID, :].astype(bf16), W3[HID:, :].astype(bf16),
    ], axis=1))

    # indicator rows for the K=128(padded) C-fold matmul: row p in {0,1}
    # covers nodes of graph 2ci+p within a 512-node chunk
    ind = np.zeros((128, 512), np.float32)
    ind[0, :256] = 1.0
    ind[1, 256:] = 1.0

    in_maps = []
    for c in range(NCORES):
        Cc = C[c * GPC:(c + 1) * GPC]                       # [64, 256]
        bpk = np.concatenate([
            b2v[:HID, None], b2v[HID:, None],
            np.ascontiguousarray(Cc[:, :HID].T),
            np.ascontiguousarray(Cc[:, HID:].T),
            np.full((HID, 1), C5[0], np.float32),
            np.full((HID, 1), C7[0], np.float32),
        ], axis=1)
        # cpk: [2, 512] indicator, then per 2-graph chunk ci the two C-m1
        # rows as a [2, 128] block
        cm1 = Cc[:, HID:].reshape(GPC // 2, 2, HID)          # [32, 2, 128]
        cblk = np.zeros((128, (GPC // 2) * HID), np.float32)
        cblk[0:2] = cm1.transpose(1, 0, 2).reshape(2, -1)
        cpk = np.concatenate([ind, cblk], axis=1)
        m = {
            "wpk": wpk,
            "bpk": np.ascontiguousarray(bpk),
            "cpk": np.ascontiguousarray(cpk).astype(bf16),
            "hT": np.ascontiguousarray(h[c * NPC:(c + 1) * NPC].T).astype(bf16),
        }
        in_maps.append(m)

    trace = bool(int(os.environ.get("KERNEL_TRACE", "0")))
    res = run_bass_kernel_spmd(nc, in_maps, core_ids=list(range(NCORES)),
                               trace=trace)
    LAST_RESULT = res

    out = np.empty((B, NPG * 2), np.float32)
    for c in range(NCORES):
        od = res.results[c]["out"]          # [98, 4096] bf16; rows 32*jj+cc live
        sel = od[[0, 1, 32, 33, 64, 65, 96, 97]].astype(np.float32)
        # sel[2*jj + cc, blk*512 + k] = edges[cc, blk*2048 + jj*512 + k]
        e = sel.reshape(4, 2, NPC // 2048, 512).transpose(1, 2, 0, 3).reshape(2, NPC)
        blk = e.reshape(2, GPC, NPG).transpose(1, 2, 0).reshape(GPC, NPG * 2)
        out[c * GPC:(c + 1) * GPC] = blk
    out += np.tile(b3v, NPG)[None, :]
    return out


# revision 20
# speedup vs baseline: 1.1738x; 1.0347x over previous
"""Trainium2 Bass kernel for nn_Batch_Edge (gnn_message_passing).

Computation (see reference):
    node_embed = last_node_batch @ W_embed + b_embed          # [B, H]
    stack      = concat([h, node_embed[seg]], axis=1)         # [N, 2H]
    out        = tanh(stack @ W1 + b1); out = tanh(out @ W2 + b2)
    edges      = out @ W3 + b3                                # [N, 2]
    result     = edges reshaped to [B, max_nodes*2]  (no padding: graphs full)

Strategy: 512 graphs (131072 nodes) sharded contiguously across 8 cores.
Activations feature-on-partition ([feature, node]); h arrives pre-transposed
bf16. The per-graph embedding contribution C = node_embed @ W1[H:, :] + b1
is host-computed in fp64.

The kernel is ACT(tanh)-bound at ~0.83ns/elem, so elementwise work is split
between ACT and DVE (GpSimd cannot touch PSUM, so it cannot help):
  - ACT: tanh for the L1 m0 half (SBUF, FD2048) + all of L2 (PSUM, FD1024,
    b2 as fused bias).
  - DVE: C-add for L1-m0 (TT broadcast from PSUM) + a custom fused DVE op
    TANH5C_ANT (clamp to +-a, degree-5 odd polynomial, 8 ALU stages, ONE
    instruction at ~1ns/elem) for the L1 m1 half, reading PSUM directly.
  - PE: besides the three matmul layers, the m1-half C broadcast is folded
    into the L1 PSUM accumulation with K=2 matmuls (two C columns as
    weights x a 2x512 indicator-row constant), so no separate C-add pass
    is needed on the m1 lane.
The deg-5 poly tanh (max err 1.4e-2 vs tanh) on one quarter of activations
keeps end-to-end rel err ~1.4e-2, under the 2e-2 gate.
"""

import os
import numpy as np

B = 512
NPG = 256               # nodes per graph
N = B * NPG             # 131072
HID = 128
NCORES = 8
GPC = B // NCORES       # 64 graphs per core
NPC = N // NCORES       # 16384 nodes per core
PAD_VALUE = -10000.0

ST = 2048               # supertile: nodes handled per main-loop iteration
NST = NPC // ST         # 8 supertiles per core

# tanh(x) ~= x' * (c2 t^2 + c1 t + c0), x' = clamp(x, +-A5), t = x'^2
# (LP minimax fit incl. saturation; max abs err 1.38e-2)
A5 = 2.140
C5 = (0.9459592131482503, -0.19428502676458773, 0.01929969961895106)
# deg-7 variant for the 2-instruction chain (max abs err 8.1e-3)
A7 = 2.420
C7 = (0.9658507348190916, -0.2314770592167479, 0.036329766992213804,
      -0.0022223059386500313)

# which lanes use the DVE poly instead of ACT tanh
POLY_L1M1 = True        # L1 m1 half -> TANH5C on DVE
POLY_MODE = int(os.environ.get("POLY_MODE", "1"))
POLY_L2M1 = False       # L2 m1 half -> CLAMPB+TANH7P chain on DVE

LAST_RESULT = None      # BassKernelResults of the most recent device run
_CACHE = {}


def _numpy_ref(last_node_batch, h, W_embed, b_embed, W1, b1, W2, b2, W3, b3,
               segment_ids, max_nodes):
    """Exact host fallback (used only if inputs don't match the expected
    uniform-graph structure)."""
    lnb = np.asarray(last_node_batch, np.float32)
    h = np.asarray(h, np.float32)
    seg = np.asarray(segment_ids).astype(np.int64)
    b = lnb.shape[0]
    n = h.shape[0]
    mn = int(np.asarray(max_nodes))
    node_embed = lnb @ np.asarray(W_embed, np.float32) + np.asarray(b_embed, np.float32)
    stack = np.concatenate([h, node_embed[seg]], axis=1)
    out = np.tanh(stack @ np.asarray(W1, np.float32) + np.asarray(b1, np.float32))
    out = np.tanh(out @ np.asarray(W2, np.float32) + np.asarray(b2, np.float32))
    edges = out @ np.asarray(W3, np.float32) + np.asarray(b3, np.float32)
    counts = np.zeros(b, np.int64)
    np.add.at(counts, seg, 1)
    offsets = np.cumsum(counts) - counts
    pos = np.arange(n) - offsets[seg]
    padded = np.full((b, mn, 2), PAD_VALUE, np.float32)
    padded[seg, pos] = edges
    return padded.reshape(b, mn * 2)


def _register_ops():
    """Register the custom fused DVE ops (idempotent)."""
    if "ops" in _CACHE:
        return _CACHE["ops"]
    import concourse.dve_ops as dvo
    from concourse.dve_ops import DveOp
    from concourse.dve_spec import (Spec, Src0, C0, C1, C2, C3, Zero,
                                    minn, maxx, sq, lower, _has_src1,
                                    _spill_c3_to_src1)
    from concourse.dve_uop import DveOpSpec

    def register(name, spec):
        if name in dvo._SUB_OPCODE_FOR_NAME:
            return next(o for o in dvo.OPS if o.name == name)
        row = max(dvo._SUB_OPCODE_FOR_NAME.values()) + 1
        assert row < 0x20
        dvo._SUB_OPCODE_FOR_NAME[name] = row
        uops = lower(spec, ver="v3")
        s = DveOpSpec(name=name, opcode=row, uops=uops, rd1_en=_has_src1(spec))
        op = DveOp(name, spec, subdim=False, uops_sha={"v3": s.sha("v3")})
        dvo.OPS.append(op)
        dvo.CUSTOM_DVE_SPECS[name] = spec
        return op

    # TANH5C: y = x'*((t*c2 + c1)*t + c0), x' = clamp(x, +-a), t = x'^2
    #   s0 = a, s1 = c2, imm2 = c1, in1[P,1] = c0
    def ref_tanh5(in0, in1, c0, c1, c2):
        a = np.asarray(c0, np.float32)
        xp = np.clip(in0.astype(np.float32), -a, a)
        t = xp * xp
        cc0 = np.asarray(in1, np.float32).reshape(-1, 1)
        return xp * ((t * np.float32(c1) + np.float32(c2)) * t + cc0)

    _xc = maxx(minn(Src0, C0), Zero - C0)
    _t5 = sq(_xc)
    op5 = register("TANH5C_ANT", Spec(
        body=_spill_c3_to_src1(((_t5 * C1 + C2) * _t5 + C3) * _xc),
        reference=ref_tanh5))

    # CLAMPB: out = clamp(Src0 + C0, +-C1); s0 = bias [P,1], s1 = a
    def ref_clampb(in0, in1, c0, c1, c2):
        return np.clip(in0.astype(np.float32) + np.asarray(c0, np.float32),
                       -c1, c1)

    op_cb = register("CLAMPB_ANT", Spec(
        body=maxx(minn(Src0 + C0, C1), Zero - C1), reference=ref_clampb))

    # TANH7P: y = x*(((t*c3 + c2)*t + c1)*t + c0), t = x^2 (input preclamped)
    #   s0 = c3, s1 = c2, imm2 = c1, in1[P,1] = c0
    def ref_tanh7(in0, in1, c0, c1, c2):
        x = in0.astype(np.float32)
        t = x * x
        cc0 = np.asarray(in1, np.float32).reshape(-1, 1)
        return x * (((t * np.asarray(c0, np.float32) + np.float32(c1)) * t
                     + np.float32(c2)) * t + cc0)

    _t7 = sq(Src0)
    op7 = register("TANH7P_ANT", Spec(
        body=_spill_c3_to_src1((((_t7 * C0 + C1) * _t7 + C2) * _t7 + C3)
                               * Src0),
        reference=ref_tanh7))

    _CACHE["ops"] = (op5, op_cb, op7)
    return _CACHE["ops"]


def _build():
    """Build + compile the per-core Bass program (identical on all cores)."""
    import concourse.bacc as bacc
    import concourse.mybir as mybir
    import concourse.tile as tile

    op5, op_cb, op7 = _register_ops()

    f32 = mybir.dt.float32
    bf16 = mybir.dt.bfloat16
    Tanh = mybir.ActivationFunctionType.Tanh
    Add = mybir.AluOpType.add

    nc = bacc.Bacc("TRN2", target_bir_lowering=False, debug=False, enable_asserts=False)

    # wpk layout (free dim): w1t[0:256] w2a[256:512] w2b[512:768]
    #                        w3a[768:770] w3b[770:772]
    hT = nc.dram_tensor("hT", [128, NPC], bf16, kind="ExternalInput").ap()
    wpk = nc.dram_tensor("wpk", [128, 772], bf16, kind="ExternalInput").ap()
    # bpk columns: 0 b2a, 1 b2b, 2:66 C^T m0, 66:130 C^T m1,
    #              130 poly c0 (deg5), 131 poly c0 (deg7)
    bpk = nc.dram_tensor("bpk", [128, 132], f32, kind="ExternalInput").ap()
    # cpk rows 0-1 (padded to K=128 so the C-fold matmul uses the same PE
    # tile config as the main matmul in its accumulation group):
    # [*, 0:512] indicator rows (1 for own graph's 256 nodes),
    # [*, 512 + ci*128 : +128] = C^T m1 for chunk ci's 2 graphs; rows 2+ zero.
    cpk = nc.dram_tensor("cpk", [128, 512 + (GPC // 2) * 128], bf16,
                         kind="ExternalInput").ap()
    # out rows {32*jj + c}: [32*jj + c, st*512 + k] = edges[c, st*2048 + jj*512 + k]
    # (other rows are garbage; one wide DMA per supertile beats 4 narrow ones)
    out_d = nc.dram_tensor("out", [98, NPC // 4], bf16, kind="ExternalOutput").ap()

    with tile.TileContext(nc) as tc:
        with (
            tc.tile_pool(name="w", bufs=1) as wp,
            tc.tile_pool(name="io", bufs=2) as io,
            tc.tile_pool(name="act", bufs=2) as ac,
            tc.tile_pool(name="ps1", bufs=3, space="PSUM") as ps1,
            tc.tile_pool(name="ps2", bufs=2, space="PSUM") as ps2,
            tc.tile_pool(name="ps3", bufs=1, space="PSUM") as ps3,
        ):
            # dummy custom op on a zeroed scratch tile: preloads the DVE
            # uop table off the critical path (like the tanh table trick)
            if POLY_MODE != 0:
                t_dum = wp.tile([128, 8], bf16, tag="dum")
                nc.vector.memset(t_dum[:], 0)
                nc.vector._custom_dve(
                    op5, out=t_dum[:, 4:8], in0=t_dum[:, 0:4],
                    in1=t_dum[:, 0:1], s0=A5, s1=C5[2], imm2=C5[1],
                )
            # setup DMAs spread across idle engine queues so the head
            # isn't serialized on the SP sequencer
            s_b = wp.tile([128, 132], f32, tag="bpk")
            nc.gpsimd.dma_start(out=s_b[:], in_=bpk)
            s_cp = wp.tile([128, 512 + (GPC // 2) * 128], bf16, tag="cpk")
            nc.gpsimd.dma_start(out=s_cp[:], in_=cpk)
            # L1 weights on the scalar queue (idle until the first tanh)
            s_w = wp.tile([128, 772], bf16, tag="wpk")
            nc.scalar.dma_start(out=s_w[:, 0:256], in_=wpk[:, 0:256])
            h_tiles = {}
            t_h0 = io.tile([128, ST], bf16, tag="h")
            nc.sync.dma_start(out=t_h0[:, 0:512], in_=hT[:, 0:512])
            nc.sync.dma_start(out=t_h0[:, 512:ST], in_=hT[:, 512:ST])
            h_tiles[0] = t_h0
            nc.sync.dma_start(out=s_w[:, 256:772], in_=wpk[:, 256:772])
            s_w1t = s_w[:, 0:256]
            s_w2a = s_w[:, 256:512]
            s_w2b = s_w[:, 512:768]
            s_w3a = s_w[:, 768:770]
            s_w3b = s_w[:, 770:772]
            s_b2 = [s_b[:, 0:1], s_b[:, 1:2]]
            s_ct = [s_b[:, 2:66], s_b[:, 66:130]]
            s_c05 = s_b[:, 130:131]
            s_c07 = s_b[:, 131:132]

            for st in range(NST):
                if st in h_tiles:
                    t_h = h_tiles.pop(st)
                else:
                    t_h = io.tile([128, ST], bf16, tag="h")
                    nc.sync.dma_start(
                        out=t_h[:], in_=hT[:, st * ST:(st + 1) * ST],
                    )

                # ---- L1: z1[m] = W1[:H, m].T @ h^T ; +C per-graph ----
                # m0: DVE TT C-add -> SBUF bf16, ACT tanh FD2048.
                # m1: C folded into PSUM by a K=2 indicator matmul on PE;
                #     DVE TANH5C (fused clamp + deg-5 poly tanh, one
                #     instruction) evacuates PSUM -> bf16 SBUF directly.
                y1 = []
                for m in (0, 1):
                    y1t = ac.tile([128, ST], bf16, tag=f"y1{m}")
                    if m == 1 and POLY_L1M1 and POLY_MODE in (1, 3):
                        if POLY_MODE == 3:
                            y1x = ac.tile([128, ST], bf16, tag="y1x")
                        else:
                            y1x = None
                        for j in range(ST // 512):
                            p1 = ps1.tile([128, 512], f32, tag="ps1")
                            nc.tensor.matmul(
                                p1[:], s_w1t[:, 128:256],
                                t_h[:, 512 * j:512 * j + 512],
                                start=True, stop=False,
                            )
                            ci = st * (ST // 512) + j
                            nc.tensor.matmul(
                                p1[:], s_cp[:, 512 + ci * 128:512 + ci * 128 + 128],
                                s_cp[:, 0:512],
                                start=False, stop=True,
                            )
                            if POLY_MODE == 1:
                                nc.vector._custom_dve(
                                    op5, out=y1t[:, 512 * j:512 * j + 512],
                                    in0=p1[:], in1=s_c05,
                                    s0=A5, s1=C5[2], imm2=C5[1],
                                )
                            else:
                                nc.vector.tensor_copy(
                                    y1x[:, 512 * j:512 * j + 512], p1[:])
                        if POLY_MODE == 3:
                            nc.scalar.activation(y1t[:], y1x[:], Tanh)
                        y1.append(y1t)
                        continue
                    if m == 1 and POLY_L1M1 and POLY_MODE in (2, 4):
                        y1s = ac.tile([128, ST], bf16, tag="y1s1")
                        for j in range(ST // 512):
                            p1 = ps1.tile([128, 512], f32, tag="ps1")
                            nc.tensor.matmul(
                                p1[:], s_w1t[:, 128:256],
                                t_h[:, 512 * j:512 * j + 512],
                                start=True, stop=True,
                            )
                            g = st * (ST // NPG) + j * 2
                            nc.vector.tensor_tensor(
                                y1s[:, 512 * j:512 * j + 512]
                                .rearrange("p (a b) -> p a b", a=2),
                                p1[:].rearrange("p (a b) -> p a b", a=2),
                                s_ct[1][:, g:g + 2].broadcast_to((128, 2, 256)),
                                Add,
                            )
                        if POLY_MODE == 2:
                            nc.vector._custom_dve(
                                op5, out=y1t[:], in0=y1s[:], in1=s_c05,
                                s0=A5, s1=C5[2], imm2=C5[1],
                            )
                        else:
                            # half poly on DVE, half exact tanh on ACT
                            nc.vector._custom_dve(
                                op5, out=y1t[:, 0:1024], in0=y1s[:, 0:1024],
                                in1=s_c05, s0=A5, s1=C5[2], imm2=C5[1],
                            )
                            nc.scalar.activation(
                                y1t[:, 1024:2048], y1s[:, 1024:2048], Tanh)
                        y1.append(y1t)
                        continue
                    y1s = ac.tile([128, ST], bf16, tag=f"y1s{m}")
                    for j in range(ST // 512):
                        p1 = ps1.tile([128, 512], f32, tag="ps1")
                        nc.tensor.matmul(
                            p1[:], s_w1t[:, 128 * m:128 * m + 128],
                            t_h[:, 512 * j:512 * j + 512],
                            start=True, stop=True,
                        )
                        g = st * (ST // NPG) + j * 2
                        nc.vector.tensor_tensor(
                            y1s[:, 512 * j:512 * j + 512]
                            .rearrange("p (a b) -> p a b", a=2),
                            p1[:].rearrange("p (a b) -> p a b", a=2),
                            s_ct[m][:, g:g + 2].broadcast_to((128, 2, 256)),
                            Add,
                        )
                    if st == 0 and m == 0:
                        # first supertile: FD=512 slices so the Scalar queue
                        # saturates earlier out of the DMA head
                        for j in range(ST // 512):
                            nc.scalar.activation(
                                y1t[:, 512 * j:512 * j + 512],
                                y1s[:, 512 * j:512 * j + 512], Tanh,
                            )
                    else:
                        nc.scalar.activation(y1t[:], y1s[:], Tanh)
                    y1.append(y1t)

                # ---- L2: z2[m] = W2[:, m].T @ y1 (+b2) ----
                # m0 (and m1 unless POLY_L2M1): ACT tanh straight from PSUM
                # with b2 as fused bias, FD1024.
                y2 = []
                for m in (0, 1):
                    yt = ac.tile([128, ST], bf16, tag=f"y2{m}")
                    use_poly = (m == 1 and POLY_L2M1)
                    xc = ac.tile([128, ST], bf16, tag="xc") if use_poly else None
                    for jj in range(ST // 1024):
                        p2 = ps2.tile([128, 1024], f32, tag="ps2")
                        for j2 in (0, 1):
                            sl = 1024 * jj + 512 * j2
                            po = 512 * j2
                            nc.tensor.matmul(
                                p2[:, po:po + 512],
                                s_w2a[:, 128 * m:128 * m + 128],
                                y1[0][:, sl:sl + 512],
                                start=True, stop=False,
                            )
                            nc.tensor.matmul(
                                p2[:, po:po + 512],
                                s_w2b[:, 128 * m:128 * m + 128],
                                y1[1][:, sl:sl + 512],
                                start=False, stop=True,
                            )
                        if use_poly:
                            nc.vector._custom_dve(
                                op_cb, out=xc[:, 1024 * jj:1024 * jj + 1024],
                                in0=p2[:], in1=None, s0=s_b2[1], s1=A7,
                            )
                        else:
                            nc.scalar.activation(
                                yt[:, 1024 * jj:1024 * jj + 1024], p2[:],
                                Tanh, bias=s_b2[m],
                            )
                    if use_poly:
                        nc.vector._custom_dve(
                            op7, out=yt[:], in0=xc[:], in1=s_c07,
                            s0=C7[3], s1=C7[2], imm2=C7[1],
                        )
                    y2.append(yt)

                # ---- L3: edges^T = W3a.T @ y2a + W3b.T @ y2b (M=2) ----
                # 4-way PE column tiling: chunk jj lands in PSUM partitions
                # [32jj, 32jj+2) of ONE bank; one [98, 512] copy on the Pool
                # engine (cost is free-dim-bound) evacuates all four pairs.
                p3 = ps3.tile([128, 512], f32, tag="ps3")
                for jj in range(4):
                    nc.tensor.matmul(
                        p3[32 * jj:32 * jj + 2, :], s_w3a,
                        y2[0][:, 512 * jj:512 * jj + 512],
                        start=True, stop=False, tile_position=(0, 32 * jj),
                    )
                for jj in range(4):
                    nc.tensor.matmul(
                        p3[32 * jj:32 * jj + 2, :], s_w3b,
                        y2[1][:, 512 * jj:512 * jj + 512],
                        start=False, stop=True, tile_position=(0, 32 * jj),
                    )
                ed = io.tile([98, 512], bf16, tag="ed")
                nc.vector.tensor_copy(ed[:], p3[0:98, :])
                nc.sync.dma_start(
                    out=out_d[:, st * 512:(st + 1) * 512], in_=ed[:],
                )

    nc.compile()
    return nc


def kernel(last_node_batch, h, W_embed, b_embed, W1, b1, W2, b2, W3, b3,
           segment_ids, max_nodes):
    global LAST_RESULT
    lnb = np.asarray(last_node_batch, np.float32)
    h = np.asarray(h, np.float32)
    seg = np.asarray(segment_ids)
    mn = int(np.asarray(max_nodes))

    expected_seg = np.repeat(np.arange(B, dtype=seg.dtype), NPG)
    if not (lnb.shape == (B, HID) and h.shape == (N, HID) and mn == NPG
            and seg.shape == (N,) and np.array_equal(seg, expected_seg)):
        return _numpy_ref(last_node_batch, h, W_embed, b_embed, W1, b1, W2, b2,
                          W3, b3, segment_ids, max_nodes)

    import sys
    try:
        import antenv.axon_hooks  # noqa: F401
    except ImportError:
        # bass_utils imports this unconditionally when tracing is requested
        # (e.g. BASS_TRACE set in the environment); provide a no-op fallback
        # so tracing degrades instead of crashing.
        import types
        _m = types.ModuleType("antenv.axon_hooks")
        _m.get_axon_ntff_profile_hook = lambda: None
        _m.set_axon_ntff_profile_hook = lambda h: None
        sys.modules["antenv.axon_hooks"] = _m

    import ml_dtypes
    from concourse.bass_utils import run_bass_kernel_spmd

    bf16 = ml_dtypes.bfloat16

    if "nc" not in _CACHE:
        _CACHE["nc"] = _build()
    nc = _CACHE["nc"]

    W1 = np.asarray(W1, np.float32)
    W2 = np.asarray(W2, np.float32)
    W3 = np.asarray(W3, np.float32)
    b2v = np.asarray(b2, np.float32)
    b3v = np.asarray(b3, np.float32)

    # Per-graph contribution C = (lnb @ W_embed + b_embed) @ W1[H:] + b1,
    # computed on host in fp64.
    emb = lnb.astype(np.float64) @ np.asarray(W_embed, np.float64) \
        + np.asarray(b_embed, np.float64)
    C = (emb @ W1[HID:, :].astype(np.float64)
         + np.asarray(b1, np.float64)).astype(np.float32)   # [B, 2H]

    wpk = np.ascontiguousarray(np.concatenate([
        W1[:HID, :].astype(bf16),
        W2[:HID, :].astype(bf16), W2[HID:, :].astype(bf16),
        W3[:HID, :].astype(bf16), W3[HID:, :].astype(bf16),
    ], axis=1))

    # indicator rows for the K=128(padded) C-fold matmul: row p in {0,1}
    # covers nodes of graph 2ci+p within a 512-node chunk
    ind = np.zeros((128, 512), np.float32)
    ind[0, :256] = 1.0
    ind[1, 256:] = 1.0

    in_maps = []
    for c in range(NCORES):
        Cc = C[c * GPC:(c + 1) * GPC]                       # [64, 256]
        bpk = np.concatenate([
            b2v[:HID, None], b2v[HID:, None],
            np.ascontiguousarray(Cc[:, :HID].T),
            np.ascontiguousarray(Cc[:, HID:].T),
            np.full((HID, 1), C5[0], np.float32),
            np.full((HID, 1), C7[0], np.float32),
        ], axis=1)
        # cpk: [2, 512] indicator, then per 2-graph chunk ci the two C-m1
        # rows as a [2, 128] block
        cm1 = Cc[:, HID:].reshape(GPC // 2, 2, HID)          # [32, 2, 128]
        cblk = np.zeros((128, (GPC // 2) * HID), np.float32)
        cblk[0:2] = cm1.transpose(1, 0, 2).reshape(2, -1)
        cpk = np.concatenate([ind, cblk], axis=1)
        m = {
            "wpk": wpk,
            "bpk": np.ascontiguousarray(bpk),
            "cpk": np.ascontiguousarray(cpk).astype(bf16),
            "hT": np.ascontiguousarray(h[c * NPC:(c + 1) * NPC].T).astype(bf16),
        }
        in_maps.append(m)

    trace = bool(int(os.environ.get("KERNEL_TRACE", "0")))
    res = run_bass_kernel_spmd(nc, in_maps, core_ids=list(range(NCORES)),
                               trace=trace)
    LAST_RESULT = res

    out = np.empty((B, NPG * 2), np.float32)
    for c in range(NCORES):
        od = res.results[c]["out"]          # [98, 4096] bf16; rows 32*jj+cc live
        sel = od[[0, 1, 32, 33, 64, 65, 96, 97]].astype(np.float32)
        # sel[2*jj + cc, blk*512 + k] = edges[cc, blk*2048 + jj*512 + k]
        e = sel.reshape(4, 2, NPC // 2048, 512).transpose(1, 2, 0, 3).reshape(2, NPC)
        blk = e.reshape(2, GPC, NPG).transpose(1, 2, 0).reshape(GPC, NPG * 2)
        out[c * GPC:(c + 1) * GPC] = blk
    out += np.tile(b3v, NPG)[None, :]
    return out


# revision 23
# speedup vs baseline: 1.2571x; 1.0710x over previous
"""Trainium2 Bass kernel for nn_Batch_Edge (gnn_message_passing).

Computation (see reference):
    node_embed = last_node_batch @ W_embed + b_embed          # [B, H]
    stack      = concat([h, node_embed[seg]], axis=1)         # [N, 2H]
    out        = tanh(stack @ W1 + b1); out = tanh(out @ W2 + b2)
    edges      = out @ W3 + b3                                # [N, 2]
    result     = edges reshaped to [B, max_nodes*2]  (no padding: all graphs full)

Strategy: shard 512 graphs (131072 nodes) contiguously across 8 cores (64
graphs / 16384 nodes each). Activations are feature-on-partition
([feature, node]); the host supplies h pre-transposed in bf16 (PE streams
bf16 at 1 col/cycle @ 2.4 GHz vs ~half rate for fp32r — the single biggest
lever). The per-graph embedding contribution C = node_embed @ W1[H:, :] + b1
is computed once per core in fp32 and added to the L1 PSUM by DVE as a
per-partition broadcast; tanh runs on ACT with large free dims. L3 (edges =
W3.T @ y2, M=2) uses 4-way PE column tiling: four concurrent matmuls land in
partition pairs {0,1},{32,33},{64,65},{96,97} of one PSUM bank, evacuated by
a single [98, 512] DVE copy (DVE cost is free-dim-bound, partitions are
parallel lanes).
"""

import os
import numpy as np

B = 512
NPG = 256               # nodes per graph
N = B * NPG             # 131072
HID = 128
NCORES = 8
GPC = B // NCORES       # 64 graphs per core
NPC = N // NCORES       # 16384 nodes per core
PAD_VALUE = -10000.0

ST = 2048               # supertile: nodes handled per main-loop iteration
NST = NPC // ST         # 8 supertiles per core

LAST_RESULT = None      # BassKernelResults of the most recent device run
_CACHE = {}


def _numpy_ref(last_node_batch, h, W_embed, b_embed, W1, b1, W2, b2, W3, b3,
               segment_ids, max_nodes):
    """Exact host fallback (used only if inputs don't match the expected
    uniform-graph structure)."""
    lnb = np.asarray(last_node_batch, np.float32)
    h = np.asarray(h, np.float32)
    seg = np.asarray(segment_ids).astype(np.int64)
    b = lnb.shape[0]
    n = h.shape[0]
    mn = int(np.asarray(max_nodes))
    node_embed = lnb @ np.asarray(W_embed, np.float32) + np.asarray(b_embed, np.float32)
    stack = np.concatenate([h, node_embed[seg]], axis=1)
    out = np.tanh(stack @ np.asarray(W1, np.float32) + np.asarray(b1, np.float32))
    out = np.tanh(out @ np.asarray(W2, np.float32) + np.asarray(b2, np.float32))
    edges = out @ np.asarray(W3, np.float32) + np.asarray(b3, np.float32)
    counts = np.zeros(b, np.int64)
    np.add.at(counts, seg, 1)
    offsets = np.cumsum(counts) - counts
    pos = np.arange(n) - offsets[seg]
    padded = np.full((b, mn, 2), PAD_VALUE, np.float32)
    padded[seg, pos] = edges
    return padded.reshape(b, mn * 2)


def _build():
    """Build + compile the per-core Bass program (identical on all cores)."""
    import concourse.bacc as bacc
    import concourse.mybir as mybir
    import concourse.tile as tile

    f32 = mybir.dt.float32
    bf16 = mybir.dt.bfloat16
    Tanh = mybir.ActivationFunctionType.Tanh

    nc = bacc.Bacc("TRN2", target_bir_lowering=False, debug=False, enable_asserts=False)

    # wpk layout (free dim): w1t[0:256] w2a[256:512] w2b[512:768]
    #                        w3a[768:770] w3b[770:772]
    hT = nc.dram_tensor("hT", [128, NPC], bf16, kind="ExternalInput").ap()
    wpk = nc.dram_tensor("wpk", [128, 772], bf16, kind="ExternalInput").ap()
    # bpk columns: b2a, b2b, C^T half0 [64], C^T half1 [64]  (C host-computed)
    bpk = nc.dram_tensor("bpk", [128, 130], f32, kind="ExternalInput").ap()
    # out rows {32*jj + c}: [32*jj + c, st*512 + k] = edges[c, st*2048 + jj*512 + k]
    # (other rows are garbage; one wide DMA per supertile beats 4 narrow ones)
    out_d = nc.dram_tensor("out", [98, NPC // 4], bf16, kind="ExternalOutput").ap()

    with tile.TileContext(nc) as tc:
        with (
            tc.tile_pool(name="w", bufs=1) as wp,
            tc.tile_pool(name="io", bufs=2) as io,
            tc.tile_pool(name="act", bufs=2) as ac,
            tc.tile_pool(name="ps1", bufs=3, space="PSUM") as ps1,
            tc.tile_pool(name="ps2", bufs=2, space="PSUM") as ps2,
            tc.tile_pool(name="ps3", bufs=1, space="PSUM") as ps3,
        ):
            # biases + host-computed C first (tiny DMA); a dummy activation
            # right after preloads the tanh table set off the critical path.
            s_b = wp.tile([128, 130], f32, tag="bpk")
            nc.sync.dma_start(out=s_b[:], in_=bpk)
            # L1 weights next, then h chunk 0, then the rest
            s_w = wp.tile([128, 772], bf16, tag="wpk")
            nc.sync.dma_start(out=s_w[:, 0:256], in_=wpk[:, 0:256])
            h_tiles = {}
            t_h0 = io.tile([128, ST], bf16, tag="h")
            nc.sync.dma_start(out=t_h0[:, 0:ST // 2], in_=hT[:, 0:ST // 2])
            nc.sync.dma_start(out=t_h0[:, ST // 2:ST], in_=hT[:, ST // 2:ST])
            h_tiles[0] = t_h0
            nc.sync.dma_start(out=s_w[:, 256:772], in_=wpk[:, 256:772])
            s_w1t = s_w[:, 0:256]
            s_w2a = s_w[:, 256:512]
            s_w2b = s_w[:, 512:768]
            s_w3a = s_w[:, 768:770]
            s_w3b = s_w[:, 770:772]
            s_b2 = [s_b[:, 0:1], s_b[:, 1:2]]
            s_ct = [s_b[:, 2:66], s_b[:, 66:130]]

            for st in range(NST):
                if st in h_tiles:
                    t_h = h_tiles.pop(st)
                else:
                    t_h = io.tile([128, ST], bf16, tag="h")
                    nc.sync.dma_start(
                        out=t_h[:], in_=hT[:, st * ST:(st + 1) * ST],
                    )

                # L1: y1[m] = tanh(W1[:H, m].T @ h^T + C[m][:, g]); C-add on
                # DVE (per-graph broadcast), tanh on ACT at FD=2048 per half —
                # per-half granularity is load-bearing: L2's m=0 matmuls
                # start while half 1 is still in flight.
                if st == 0:
                    # interleave m0/m1 chunks so the m1 half (which gates L2)
                    # isn't serialized behind all of m0 at the pipeline head
                    y1s_st0 = []
                    y1t_st0 = []
                    for m in (0, 1):
                        t_s = ac.tile([128, ST], bf16, tag=f"y1s{m}")
                        t_t = ac.tile([128, ST], bf16, tag=f"y1{m}")
                        y1s_st0.append(t_s)
                        y1t_st0.append(t_t)
                    for j in range(ST // 512):
                        for m in (0, 1):
                            p1 = ps1.tile([128, 512], f32, tag="ps1")
                            nc.tensor.matmul(
                                p1[:], s_w1t[:, 128 * m:128 * m + 128],
                                t_h[:, 512 * j:512 * j + 512],
                                start=True, stop=True,
                            )
                            g = st * (ST // NPG) + j * 2
                            nc.vector.tensor_tensor(
                                y1s_st0[m][:, 512 * j:512 * j + 512]
                                .rearrange("p (a b) -> p a b", a=2),
                                p1[:].rearrange("p (a b) -> p a b", a=2),
                                s_ct[m][:, g:g + 2].broadcast_to((128, 2, 256)),
                                mybir.AluOpType.add,
                            )
                            nc.scalar.activation(
                                y1t_st0[m][:, 512 * j:512 * j + 512],
                                y1s_st0[m][:, 512 * j:512 * j + 512], Tanh,
                            )
                    y1 = y1t_st0
                else:
                    y1 = []
                for m in (0, 1) if st != 0 else ():
                    y1s = ac.tile([128, ST], bf16, tag=f"y1s{m}")
                    for j in range(ST // 512):
                        p1 = ps1.tile([128, 512], f32, tag="ps1")
                        nc.tensor.matmul(
                            p1[:], s_w1t[:, 128 * m:128 * m + 128],
                            t_h[:, 512 * j:512 * j + 512],
                            start=True, stop=True,
                        )
                        g = st * (ST // NPG) + j * 2
                        nc.vector.tensor_tensor(
                            y1s[:, 512 * j:512 * j + 512]
                            .rearrange("p (a b) -> p a b", a=2),
                            p1[:].rearrange("p (a b) -> p a b", a=2),
                            s_ct[m][:, g:g + 2].broadcast_to((128, 2, 256)),
                            mybir.AluOpType.add,
                        )
                    y1t = ac.tile([128, ST], bf16, tag=f"y1{m}")
                    if st == 0:
                        # first supertile: FD=512 slices behind each TT so the
                        # Scalar queue saturates earlier out of the DMA head
                        for j in range(ST // 512):
                            nc.scalar.activation(
                                y1t[:, 512 * j:512 * j + 512],
                                y1s[:, 512 * j:512 * j + 512], Tanh,
                            )
                    else:
                        nc.scalar.activation(y1t[:], y1s[:], Tanh)
                    y1.append(y1t)

                # L2: y2[m] = tanh(W2[:, m].T @ y1 + b2[m]); tanh reads the
                # [128, 1024] PSUM tile directly.
                y2 = []
                for m in (0, 1):
                    yt = ac.tile([128, ST], bf16, tag=f"y2{m}")
                    for jj in range(ST // 1024):
                        p2 = ps2.tile([128, 1024], f32, tag="ps2")
                        for j2 in (0, 1):
                            sl = 1024 * jj + 512 * j2
                            po = 512 * j2
                            nc.tensor.matmul(
                                p2[:, po:po + 512],
                                s_w2a[:, 128 * m:128 * m + 128],
                                y1[0][:, sl:sl + 512],
                                start=True, stop=False,
                            )
                            nc.tensor.matmul(
                                p2[:, po:po + 512],
                                s_w2b[:, 128 * m:128 * m + 128],
                                y1[1][:, sl:sl + 512],
                                start=False, stop=True,
                            )
                        nc.scalar.activation(
                            yt[:, 1024 * jj:1024 * jj + 1024], p2[:],
                            Tanh, bias=s_b2[m],
                        )
                    y2.append(yt)

                # L3: edges^T = W3a.T @ y2a + W3b.T @ y2b (M=2). 4-way PE
                # column tiling: chunk jj lands in PSUM partitions
                # [32jj, 32jj+2) of ONE bank; the four matmuls per round run
                # concurrently on disjoint col-groups. One [98, 512] DVE copy
                # evacuates all four pairs (cost is free-dim-bound); the four
                # output DMAs go out on the idle GpSimd queue.
                p3 = ps3.tile([128, 512], f32, tag="ps3")
                for jj in range(4):
                    nc.tensor.matmul(
                        p3[32 * jj:32 * jj + 2, :], s_w3a,
                        y2[0][:, 512 * jj:512 * jj + 512],
                        start=True, stop=False, tile_position=(0, 32 * jj),
                    )
                for jj in range(4):
                    nc.tensor.matmul(
                        p3[32 * jj:32 * jj + 2, :], s_w3b,
                        y2[1][:, 512 * jj:512 * jj + 512],
                        start=False, stop=True, tile_position=(0, 32 * jj),
                    )
                ed = io.tile([98, 512], bf16, tag="ed")
                nc.vector.tensor_copy(ed[:], p3[0:98, :])
                nc.sync.dma_start(
                    out=out_d[:, st * 512:(st + 1) * 512], in_=ed[:],
                )

    nc.compile()
    return nc


def kernel(last_node_batch, h, W_embed, b_embed, W1, b1, W2, b2, W3, b3,
           segment_ids, max_nodes):
    global LAST_RESULT
    lnb = np.asarray(last_node_batch, np.float32)
    h = np.asarray(h, np.float32)
    seg = np.asarray(segment_ids)
    mn = int(np.asarray(max_nodes))

    expected_seg = np.repeat(np.arange(B, dtype=seg.dtype), NPG)
    if not (lnb.shape == (B, HID) and h.shape == (N, HID) and mn == NPG
            and seg.shape == (N,) and np.array_equal(seg, expected_seg)):
        return _numpy_ref(last_node_batch, h, W_embed, b_embed, W1, b1, W2, b2,
                          W3, b3, segment_ids, max_nodes)

    import sys
    try:
        import antenv.axon_hooks  # noqa: F401
    except ImportError:
        # bass_utils imports this unconditionally when tracing is requested
        # (e.g. BASS_TRACE set in the environment); provide a no-op fallback
        # so tracing degrades instead of crashing.
        import types
        _m = types.ModuleType("antenv.axon_hooks")
        _m.get_axon_ntff_profile_hook = lambda: None
        _m.set_axon_ntff_profile_hook = lambda h: None
        sys.modules["antenv.axon_hooks"] = _m

    import ml_dtypes
    from concourse.bass_utils import run_bass_kernel_spmd

    bf16 = ml_dtypes.bfloat16

    if "nc" not in _CACHE:
        _CACHE["nc"] = _build()
    nc = _CACHE["nc"]

    W1 = np.asarray(W1, np.float32)
    W2 = np.asarray(W2, np.float32)
    W3 = np.asarray(W3, np.float32)
    b2v = np.asarray(b2, np.float32)
    b3v = np.asarray(b3, np.float32)

    # Per-graph contribution C = (lnb @ W_embed + b_embed) @ W1[H:] + b1,
    # computed on host in fp64 (more accurate than the old device bf16 path).
    emb = lnb.astype(np.float64) @ np.asarray(W_embed, np.float64) \
        + np.asarray(b_embed, np.float64)
    C = (emb @ W1[HID:, :].astype(np.float64)
         + np.asarray(b1, np.float64)).astype(np.float32)   # [B, 2H]

    wpk = np.ascontiguousarray(np.concatenate([
        W1[:HID, :].astype(bf16),
        W2[:HID, :].astype(bf16), W2[HID:, :].astype(bf16),
        W3[:HID, :].astype(bf16), W3[HID:, :].astype(bf16),
    ], axis=1))

    in_maps = []
    for c in range(NCORES):
        Cc = C[c * GPC:(c + 1) * GPC]                       # [64, 256]
        bpk = np.concatenate([
            b2v[:HID, None], b2v[HID:, None],
            np.ascontiguousarray(Cc[:, :HID].T),
            np.ascontiguousarray(Cc[:, HID:].T),
        ], axis=1)
        m = {
            "wpk": wpk,
            "bpk": np.ascontiguousarray(bpk),
            "hT": np.ascontiguousarray(h[c * NPC:(c + 1) * NPC].T).astype(bf16),
        }
        in_maps.append(m)

    trace = bool(int(os.environ.get("KERNEL_TRACE", "0")))
    res = run_bass_kernel_spmd(nc, in_maps, core_ids=list(range(NCORES)),
                               trace=trace)
    LAST_RESULT = res

    out = np.empty((B, NPG * 2), np.float32)
    for c in range(NCORES):
        od = res.results[c]["out"]          # [98, 4096] bf16; rows 32*jj+cc live
        sel = od[[0, 1, 32, 33, 64, 65, 96, 97]].astype(np.float32)
        # sel[2*jj + cc, blk*512 + k] = edges[cc, blk*2048 + jj*512 + k]
        e = sel.reshape(4, 2, NPC // 2048, 512).transpose(1, 2, 0, 3).reshape(2, NPC)
        blk = e.reshape(2, GPC, NPG).transpose(1, 2, 0).reshape(GPC, NPG * 2)
        out[c * GPC:(c + 1) * GPC] = blk
    out += np.tile(b3v, NPG)[None, :]
    return out



# revision 24
# speedup vs baseline: 1.2625x; 1.0043x over previous
"""Trainium2 Bass kernel for nn_Batch_Edge (gnn_message_passing).

Computation (see reference):
    node_embed = last_node_batch @ W_embed + b_embed          # [B, H]
    stack      = concat([h, node_embed[seg]], axis=1)         # [N, 2H]
    out        = tanh(stack @ W1 + b1); out = tanh(out @ W2 + b2)
    edges      = out @ W3 + b3                                # [N, 2]
    result     = edges reshaped to [B, max_nodes*2]  (no padding: all graphs full)

Strategy: shard 512 graphs (131072 nodes) contiguously across 8 cores (64
graphs / 16384 nodes each). Activations are feature-on-partition
([feature, node]); the host supplies h pre-transposed in bf16 (PE streams
bf16 at 1 col/cycle @ 2.4 GHz vs ~half rate for fp32r — the single biggest
lever). The per-graph embedding contribution C = node_embed @ W1[H:, :] + b1
is computed once per core in fp32 and added to the L1 PSUM by DVE as a
per-partition broadcast; tanh runs on ACT with large free dims. L3 (edges =
W3.T @ y2, M=2) uses 4-way PE column tiling: four concurrent matmuls land in
partition pairs {0,1},{32,33},{64,65},{96,97} of one PSUM bank, evacuated by
a single [98, 512] DVE copy (DVE cost is free-dim-bound, partitions are
parallel lanes).
"""

import os
import numpy as np

B = 512
NPG = 256               # nodes per graph
N = B * NPG             # 131072
HID = 128
NCORES = 8
GPC = B // NCORES       # 64 graphs per core
NPC = N // NCORES       # 16384 nodes per core
PAD_VALUE = -10000.0

ST = 2048               # supertile: nodes handled per main-loop iteration
NST = NPC // ST         # 8 supertiles per core

LAST_RESULT = None      # BassKernelResults of the most recent device run
_CACHE = {}


def _numpy_ref(last_node_batch, h, W_embed, b_embed, W1, b1, W2, b2, W3, b3,
               segment_ids, max_nodes):
    """Exact host fallback (used only if inputs don't match the expected
    uniform-graph structure)."""
    lnb = np.asarray(last_node_batch, np.float32)
    h = np.asarray(h, np.float32)
    seg = np.asarray(segment_ids).astype(np.int64)
    b = lnb.shape[0]
    n = h.shape[0]
    mn = int(np.asarray(max_nodes))
    node_embed = lnb @ np.asarray(W_embed, np.float32) + np.asarray(b_embed, np.float32)
    stack = np.concatenate([h, node_embed[seg]], axis=1)
    out = np.tanh(stack @ np.asarray(W1, np.float32) + np.asarray(b1, np.float32))
    out = np.tanh(out @ np.asarray(W2, np.float32) + np.asarray(b2, np.float32))
    edges = out @ np.asarray(W3, np.float32) + np.asarray(b3, np.float32)
    counts = np.zeros(b, np.int64)
    np.add.at(counts, seg, 1)
    offsets = np.cumsum(counts) - counts
    pos = np.arange(n) - offsets[seg]
    padded = np.full((b, mn, 2), PAD_VALUE, np.float32)
    padded[seg, pos] = edges
    return padded.reshape(b, mn * 2)


def _build():
    """Build + compile the per-core Bass program (identical on all cores)."""
    import concourse.bacc as bacc
    import concourse.mybir as mybir
    import concourse.tile as tile

    f32 = mybir.dt.float32
    bf16 = mybir.dt.bfloat16
    Tanh = mybir.ActivationFunctionType.Tanh

    nc = bacc.Bacc("TRN2", target_bir_lowering=False, debug=False, enable_asserts=False)

    # wpk layout (free dim): w1t[0:256] w2a[256:512] w2b[512:768]
    #                        w3a[768:770] w3b[770:772]
    hT = nc.dram_tensor("hT", [128, NPC], bf16, kind="ExternalInput").ap()
    wpk = nc.dram_tensor("wpk", [128, 772], bf16, kind="ExternalInput").ap()
    # bpk columns: b2a, b2b, C^T half0 [64], C^T half1 [64]  (C host-computed)
    bpk = nc.dram_tensor("bpk", [128, 130], f32, kind="ExternalInput").ap()
    # out rows {32*jj + c}: [32*jj + c, st*512 + k] = edges[c, st*2048 + jj*512 + k]
    # (other rows are garbage; one wide DMA per supertile beats 4 narrow ones)
    out_d = nc.dram_tensor("out", [98, NPC // 4], bf16, kind="ExternalOutput").ap()

    with tile.TileContext(nc) as tc:
        with (
            tc.tile_pool(name="w", bufs=1) as wp,
            tc.tile_pool(name="io", bufs=2) as io,
            tc.tile_pool(name="act", bufs=2) as ac,
            tc.tile_pool(name="ps1", bufs=3, space="PSUM") as ps1,
            tc.tile_pool(name="ps2", bufs=2, space="PSUM") as ps2,
            tc.tile_pool(name="ps3", bufs=1, space="PSUM") as ps3,
        ):
            # biases + host-computed C first (tiny DMA); a dummy activation
            # right after preloads the tanh table set off the critical path.
            s_b = wp.tile([128, 130], f32, tag="bpk")
            nc.sync.dma_start(out=s_b[:], in_=bpk)
            # L1 weights next, then h chunk 0, then the rest
            s_w = wp.tile([128, 772], bf16, tag="wpk")
            nc.sync.dma_start(out=s_w[:, 0:256], in_=wpk[:, 0:256])
            h_tiles = {}
            t_h0 = io.tile([128, ST], bf16, tag="h")
            nc.sync.dma_start(out=t_h0[:, 0:ST // 2], in_=hT[:, 0:ST // 2])
            nc.sync.dma_start(out=t_h0[:, ST // 2:ST], in_=hT[:, ST // 2:ST])
            h_tiles[0] = t_h0
            nc.sync.dma_start(out=s_w[:, 256:772], in_=wpk[:, 256:772])
            s_w1t = s_w[:, 0:256]
            s_w2a = s_w[:, 256:512]
            s_w2b = s_w[:, 512:768]
            s_w3a = s_w[:, 768:770]
            s_w3b = s_w[:, 770:772]
            s_b2 = [s_b[:, 0:1], s_b[:, 1:2]]
            s_ct = [s_b[:, 2:66], s_b[:, 66:130]]

            for st in range(NST):
                if st in h_tiles:
                    t_h = h_tiles.pop(st)
                else:
                    t_h = io.tile([128, ST], bf16, tag="h")
                    nc.sync.dma_start(
                        out=t_h[:], in_=hT[:, st * ST:(st + 1) * ST],
                    )

                # L1: y1[m] = tanh(W1[:H, m].T @ h^T + C[m][:, g]); C-add on
                # DVE (per-graph broadcast), tanh on ACT at FD=2048 per half —
                # per-half granularity is load-bearing: L2's m=0 matmuls
                # start while half 1 is still in flight.
                if st == 0:
                    # interleave m0/m1 chunks so the m1 half (which gates L2)
                    # isn't serialized behind all of m0 at the pipeline head
                    y1s_st0 = []
                    y1t_st0 = []
                    for m in (0, 1):
                        t_s = ac.tile([128, ST], bf16, tag=f"y1s{m}")
                        t_t = ac.tile([128, ST], bf16, tag=f"y1{m}")
                        y1s_st0.append(t_s)
                        y1t_st0.append(t_t)
                    for j in range(ST // 512):
                        for m in (0, 1):
                            p1 = ps1.tile([128, 512], f32, tag="ps1")
                            nc.tensor.matmul(
                                p1[:], s_w1t[:, 128 * m:128 * m + 128],
                                t_h[:, 512 * j:512 * j + 512],
                                start=True, stop=True,
                            )
                            g = st * (ST // NPG) + j * 2
                            nc.vector.tensor_tensor(
                                y1s_st0[m][:, 512 * j:512 * j + 512]
                                .rearrange("p (a b) -> p a b", a=2),
                                p1[:].rearrange("p (a b) -> p a b", a=2),
                                s_ct[m][:, g:g + 2].broadcast_to((128, 2, 256)),
                                mybir.AluOpType.add,
                            )
                            nc.scalar.activation(
                                y1t_st0[m][:, 512 * j:512 * j + 512],
                                y1s_st0[m][:, 512 * j:512 * j + 512], Tanh,
                            )
                    y1 = y1t_st0
                else:
                    y1 = []
                for m in (0, 1) if st != 0 else ():
                    y1s = ac.tile([128, ST], bf16, tag=f"y1s{m}")
                    for j in range(ST // 512):
                        p1 = ps1.tile([128, 512], f32, tag="ps1")
                        nc.tensor.matmul(
                            p1[:], s_w1t[:, 128 * m:128 * m + 128],
                            t_h[:, 512 * j:512 * j + 512],
                            start=True, stop=True,
                        )
                        g = st * (ST // NPG) + j * 2
                        nc.vector.tensor_tensor(
                            y1s[:, 512 * j:512 * j + 512]
                            .rearrange("p (a b) -> p a b", a=2),
                            p1[:].rearrange("p (a b) -> p a b", a=2),
                            s_ct[m][:, g:g + 2].broadcast_to((128, 2, 256)),
                            mybir.AluOpType.add,
                        )
                    y1t = ac.tile([128, ST], bf16, tag=f"y1{m}")
                    if st == 0:
                        # first supertile: FD=512 slices behind each TT so the
                        # Scalar queue saturates earlier out of the DMA head
                        for j in range(ST // 512):
                            nc.scalar.activation(
                                y1t[:, 512 * j:512 * j + 512],
                                y1s[:, 512 * j:512 * j + 512], Tanh,
                            )
                    else:
                        nc.scalar.activation(y1t[:], y1s[:], Tanh)
                    y1.append(y1t)

                # L2: y2[m] = tanh(W2[:, m].T @ y1 + b2[m]); tanh reads the
                # [128, 1024] PSUM tile directly.
                y2 = []
                for m in (0, 1):
                    yt = ac.tile([128, ST], bf16, tag=f"y2{m}")
                    for jj in range(ST // 1024):
                        p2 = ps2.tile([128, 1024], f32, tag="ps2")
                        for j2 in (0, 1):
                            sl = 1024 * jj + 512 * j2
                            po = 512 * j2
                            nc.tensor.matmul(
                                p2[:, po:po + 512],
                                s_w2a[:, 128 * m:128 * m + 128],
                                y1[0][:, sl:sl + 512],
                                start=True, stop=False,
                            )
                            nc.tensor.matmul(
                                p2[:, po:po + 512],
                                s_w2b[:, 128 * m:128 * m + 128],
                                y1[1][:, sl:sl + 512],
                                start=False, stop=True,
                            )
                        nc.scalar.activation(
                            yt[:, 1024 * jj:1024 * jj + 1024], p2[:],
                            Tanh, bias=s_b2[m],
                        )
                    y2.append(yt)

                # L3: edges^T = W3a.T @ y2a + W3b.T @ y2b (M=2). 4-way PE
                # column tiling: chunk jj lands in PSUM partitions
                # [32jj, 32jj+2) of ONE bank; the four matmuls per round run
                # concurrently on disjoint col-groups. One [98, 512] DVE copy
                # evacuates all four pairs (cost is free-dim-bound); the four
                # output DMAs go out on the idle GpSimd queue.
                p3 = ps3.tile([128, 512], f32, tag="ps3")
                if st == NST - 1:
                    # last supertile: drain L3 in two halves so the first
                    # half's evacuation overlaps the final ACT instructions
                    ed = io.tile([98, 512], bf16, tag="ed")
                    for half in (0, 1):
                        for jj in (2 * half, 2 * half + 1):
                            nc.tensor.matmul(
                                p3[32 * jj:32 * jj + 2, :], s_w3a,
                                y2[0][:, 512 * jj:512 * jj + 512],
                                start=True, stop=False,
                                tile_position=(0, 32 * jj),
                            )
                            nc.tensor.matmul(
                                p3[32 * jj:32 * jj + 2, :], s_w3b,
                                y2[1][:, 512 * jj:512 * jj + 512],
                                start=False, stop=True,
                                tile_position=(0, 32 * jj),
                            )
                        lo, hi = 64 * half, 64 * half + 66 if half else 34
                        lo = 64 * half
                        n_par = 34 if half == 0 else 34
                        nc.vector.tensor_copy(
                            ed[64 * half:64 * half + 34, :],
                            p3[64 * half:64 * half + 34, :])
                        nc.sync.dma_start(
                            out=out_d[64 * half:64 * half + 34,
                                      st * 512:(st + 1) * 512],
                            in_=ed[64 * half:64 * half + 34, :],
                        )
                else:
                    for jj in range(4):
                        nc.tensor.matmul(
                            p3[32 * jj:32 * jj + 2, :], s_w3a,
                            y2[0][:, 512 * jj:512 * jj + 512],
                            start=True, stop=False, tile_position=(0, 32 * jj),
                        )
                    for jj in range(4):
                        nc.tensor.matmul(
                            p3[32 * jj:32 * jj + 2, :], s_w3b,
                            y2[1][:, 512 * jj:512 * jj + 512],
                            start=False, stop=True, tile_position=(0, 32 * jj),
                        )
                    ed = io.tile([98, 512], bf16, tag="ed")
                    nc.vector.tensor_copy(ed[:], p3[0:98, :])
                    nc.sync.dma_start(
                        out=out_d[:, st * 512:(st + 1) * 512], in_=ed[:],
                    )

    nc.compile()
    return nc


def kernel(last_node_batch, h, W_embed, b_embed, W1, b1, W2, b2, W3, b3,
           segment_ids, max_nodes):
    global LAST_RESULT
    lnb = np.asarray(last_node_batch, np.float32)
    h = np.asarray(h, np.float32)
    seg = np.asarray(segment_ids)
    mn = int(np.asarray(max_nodes))

    expected_seg = np.repeat(np.arange(B, dtype=seg.dtype), NPG)
    if not (lnb.shape == (B, HID) and h.shape == (N, HID) and mn == NPG
            and seg.shape == (N,) and np.array_equal(seg, expected_seg)):
        return _numpy_ref(last_node_batch, h, W_embed, b_embed, W1, b1, W2, b2,
                          W3, b3, segment_ids, max_nodes)

    import sys
    try:
        import antenv.axon_hooks  # noqa: F401
    except ImportError:
        # bass_utils imports this unconditionally when tracing is requested
        # (e.g. BASS_TRACE set in the environment); provide a no-op fallback
        # so tracing degrades instead of crashing.
        import types
        _m = types.ModuleType("antenv.axon_hooks")
        _m.get_axon_ntff_profile_hook = lambda: None
        _m.set_axon_ntff_profile_hook = lambda h: None
        sys.modules["antenv.axon_hooks"] = _m

    import ml_dtypes
    from concourse.bass_utils import run_bass_kernel_spmd

    bf16 = ml_dtypes.bfloat16

    if "nc" not in _CACHE:
        _CACHE["nc"] = _build()
    nc = _CACHE["nc"]

    W1 = np.asarray(W1, np.float32)
    W2 = np.asarray(W2, np.float32)
    W3 = np.asarray(W3, np.float32)
    b2v = np.asarray(b2, np.float32)
    b3v = np.asarray(b3, np.float32)

    # Per-graph contribution C = (lnb @ W_embed + b_embed) @ W1[H:] + b1,
    # computed on host in fp64 (more accurate than the old device bf16 path).
    emb = lnb.astype(np.float64) @ np.asarray(W_embed, np.float64) \
        + np.asarray(b_embed, np.float64)
    C = (emb @ W1[HID:, :].astype(np.float64)
         + np.asarray(b1, np.float64)).astype(np.float32)   # [B, 2H]

    wpk = np.ascontiguousarray(np.concatenate([
        W1[:HID, :].astype(bf16),
        W2[:HID, :].astype(bf16), W2[HID:, :].astype(bf16),
        W3[:HID, :].astype(bf16), W3[HID:, :].astype(bf16),
    ], axis=1))

    in_maps = []
    for c in range(NCORES):
        Cc = C[c * GPC:(c + 1) * GPC]                       # [64, 256]
        bpk = np.concatenate([
            b2v[:HID, None], b2v[HID:, None],
            np.ascontiguousarray(Cc[:, :HID].T),
            np.ascontiguousarray(Cc[:, HID:].T),
        ], axis=1)
        m = {
            "wpk": wpk,
            "bpk": np.ascontiguousarray(bpk),
            "hT": np.ascontiguousarray(h[c * NPC:(c + 1) * NPC].T).astype(bf16),
        }
        in_maps.append(m)

    trace = bool(int(os.environ.get("KERNEL_TRACE", "0")))
    res = run_bass_kernel_spmd(nc, in_maps, core_ids=list(range(NCORES)),
                               trace=trace)
    LAST_RESULT = res

    out = np.empty((B, NPG * 2), np.float32)
    for c in range(NCORES):
        od = res.results[c]["out"]          # [98, 4096] bf16; rows 32*jj+cc live
        sel = od[[0, 1, 32, 33, 64, 65, 96, 97]].astype(np.float32)
        # sel[2*jj + cc, blk*512 + k] = edges[cc, blk*2048 + jj*512 + k]
        e = sel.reshape(4, 2, NPC // 2048, 512).transpose(1, 2, 0, 3).reshape(2, NPC)
        blk = e.reshape(2, GPC, NPG).transpose(1, 2, 0).reshape(GPC, NPG * 2)
        out[c * GPC:(c + 1) * GPC] = blk
    out += np.tile(b3v, NPG)[None, :]
    return out

